# revision 38
# baseline (speedup 1.0000x reference)
"""NlmCNN (weight-predicting CNN + per-pixel 13x13 weighted sum) on 8 trn2 cores.

Sharding: data-parallel over batch (8 images -> 8 cores), weights replicated.

Per-core layout trick: output y is the conv stack's result center-cropped by
6 pixels, and the receptive field of the three 3x3 convs only reaches 3 px
out, so SAME-padding never materializes: every layer is computed VALID-style
on an unpadded 256-stride flat layout. Column-wrap junk from flat shifted
reads stays confined to the outer <=3 columns of each layer, which are
discarded by the crop.

All matmul operands are bf16 (fp32 PSUM accumulation; end-to-end absmax-rel
~4e-3 vs the 2e-2 gate). bf16 is chosen over float32r because fp32-class
LDWEIGHTS runs ~2.2ns/stationary-column with FWL disabled: M=128 weight
loads (285ns) exceed the N=512 stream time (213ns) and the PE becomes
weight-load-bound (measured 426ns/matmul cadence). bf16 enables FWL and
streams the same 1 column/cycle.

Pipeline per strip of S output rows (strips software-pipelined: conv1 of
strip i+1 is emitted during strip i's conv3 phase):
  conv1: per-2-chunk im2col [9, 1024] via one 3-dim DMA -> K=9 matmul; all
         of a strip's im2col DMAs are issued a strip ahead (the imc pool
         holds a full strip) so the PE never waits on DMA latency.
  conv2/conv3: 3x3 taps packed into K=128 pair-matmuls: taps (du,0)+(du,1)
         pair on hA=[h; h<<1] (upper 64 partitions hold h shifted +1);
         taps (0,2)+(1,2) pair on hB=[h; h<<W]; only tap (2,2) is a K=64
         single. 5 matmuls per 512-px chunk for conv2, 10 for conv3
         (out channels split [0:128] M=128 / [128:169] M=41). The shifted
         tiles are built by SBUF->SBUF dup DMAs per half-strip. Chunk
         parity alternates [single | pairs] / [pairs | single] so the PE
         sees one K-row-size transition per chunk.
  einsum: patch matrix xs[t, s] = x[pos + shift(t)] split [128 | 41] taps,
         gathered bf16 by one contiguous DMA per tap-row u (row u=9 is
         split 11/2 across the two tiles); DVE scalar_tensor_tensor
         computes t2 = (conv3_lo + b3_lo) * xs_lo and
         t_hi = (conv3_hi + b3_hi) * xs_hi straight from PSUM; the
         partition reduction is "staircase ones" matmuls (K=128 over t2
         next to the pairs, K=41 over t_hi next to the K=64 singles --
         both transition-free), accumulating 2-row chunk jj into row jj
         of a persistent PSUM tile; one copy + one DMA store the image.
"""

import numpy as np

import concourse.bacc as bacc
import concourse.bass as bass
import concourse.mybir as mybir
import concourse.tile as tile
from concourse.bass_utils import run_bass_kernel_spmd

F32 = mybir.dt.float32
BF16 = mybir.dt.bfloat16
AF = mybir.ActivationFunctionType
ALU = mybir.AluOpType

H = 256
W = 256
K = 13
HO = H - K + 1  # 244
CH = 64
C3 = K * K  # 169
CLO = 128   # conv3 out-channel group sizes
CHI = C3 - 128  # 41
S_STRIP = 16
NC_ = 512  # chunk positions (2 image rows)
import os
PIPE = os.environ.get("K_PIPE", "1") == "1"      # cross-strip sw pipelining
XS_GP = os.environ.get("K_XS_GP", "1") == "1"    # xs DMAs on gpsimd queue
DUP_GP = os.environ.get("K_DUP_GP", "1") == "1"  # dup DMAs on gpsimd queue


def _ap(t, off, dims):
    return bass.AP(t, off, [list(d) for d in dims])


def _mm(nc, out, lhsT, rhs, start, stop):
    nc.tensor.matmul(out, lhsT, rhs, start=start, stop=stop)


def build_nc():
    nc = bacc.Bacc("TRN2", target_bir_lowering=False, debug=False)

    x = nc.dram_tensor("x", [1, 1, H, W], F32, kind="ExternalInput")
    w1 = nc.dram_tensor("W1", [CH, 1, 3, 3], F32, kind="ExternalInput")
    b1 = nc.dram_tensor("b1", [CH], F32, kind="ExternalInput")
    w2 = nc.dram_tensor("W2", [CH, CH, 3, 3], F32, kind="ExternalInput")
    b2 = nc.dram_tensor("b2", [CH], F32, kind="ExternalInput")
    w3 = nc.dram_tensor("W3", [C3, CH, 3, 3], F32, kind="ExternalInput")
    b3 = nc.dram_tensor("b3", [C3], F32, kind="ExternalInput")
    y = nc.dram_tensor("y", [1, 1, HO, HO], F32, kind="ExternalOutput")
    xb = nc.dram_tensor("x_b", [H * W], BF16)

    with tile.TileContext(nc) as tc:
        Body(nc, tc, x, w1, b1, w2, b2, w3, b3, y, xb).build()

    nc.compile()
    return nc


class Body:
    def __init__(self, nc, tc, x, w1, b1, w2, b2, w3, b3, y, xb):
        self.nc, self.tc = nc, tc
        self.x, self.w1, self.b1, self.w2, self.b2 = x, w1, b1, w2, b2
        self.w3, self.b3, self.y, self.xb = w3, b3, y, xb

    def build(self):
        nc, tc = self.nc, self.tc
        with (
            tc.tile_pool(name="consts", bufs=1) as consts,
            tc.tile_pool(name="t2p", bufs=3) as p_t2,
            tc.tile_pool(name="thp", bufs=3) as p_th,
            tc.tile_pool(name="imc", bufs=6) as p_imc,
            tc.tile_pool(name="h1p", bufs=2) as p_h1,
            tc.tile_pool(name="h1bp", bufs=2) as p_h1b,
            tc.tile_pool(name="h2p", bufs=2) as p_h2,
            tc.tile_pool(name="h2bp", bufs=2) as p_h2b,
            tc.tile_pool(name="xsl", bufs=2) as p_xsl,
            tc.tile_pool(name="xsh", bufs=2) as p_xsh,
            tc.tile_pool(name="yout", bufs=1) as p_y,
            tc.tile_pool(name="ps12", bufs=3, space="PSUM") as ps12,
            tc.tile_pool(name="ps3", bufs=2, space="PSUM") as ps3,
            tc.tile_pool(name="psy", bufs=1, space="PSUM") as psy,
        ):
            self.consts = consts
            self.p_t2, self.p_th, self.p_imc = p_t2, p_th, p_imc
            self.p_h1, self.p_h1b = p_h1, p_h1b
            self.p_h2, self.p_h2b = p_h2, p_h2b
            self.p_xsl, self.p_xsh = p_xsl, p_xsh
            self.p_y, self.ps12, self.ps3, self.psy = p_y, ps12, ps3, psy
            self._build_consts()
            self._build_strips()

    def _build_consts(self):
        nc, tc, consts = self.nc, self.tc, self.consts
        stage = tc.alloc_tile_pool(name="stage", bufs=1)
        # weight-prep transposes borrow ps3's "ps3lo" slot (same max tile
        # size, consts-time only) so no dedicated PSUM bank is needed
        pwtr = self.ps3

        # Weights arrive [co, ci, du, dv]; matmuls need [ci, co] per tap.
        # A strided gather DMA would be 4-byte-descriptor-bound, so load
        # contiguously and transpose on the PE instead.
        from concourse.masks import make_identity

        ident = stage.tile([128, 128], F32)
        make_identity(nc, ident[:])

        # x -> bf16 copy in DRAM (conv1 im2col + xs gather source); staged
        # FIRST so strip 0's im2col/xs DMAs can start while weight prep runs
        xst = stage.tile([128, H * W // 128], F32)
        nc.sync.dma_start(
            out=xst[:], in_=_ap(self.x, 0, [(H * W // 128, 128), (1, H * W // 128)])
        )
        xsb = stage.tile([128, H * W // 128], BF16)
        nc.vector.tensor_copy(xsb[:], xst[:])
        nc.sync.dma_start(
            out=_ap(self.xb, 0, [(H * W // 128, 128), (1, H * W // 128)]), in_=xsb[:]
        )

        w1raw = stage.tile([CH, 9], F32)
        nc.sync.dma_start(out=w1raw[:], in_=_ap(self.w1, 0, [(9, CH), (1, 9)]))
        w2raw = stage.tile([CH, 9 * CH], F32)
        nc.sync.dma_start(out=w2raw[:], in_=_ap(self.w2, 0, [(9 * CH, CH), (1, 9 * CH)]))
        w3raw_a = stage.tile([128, 9 * CH], F32)
        nc.sync.dma_start(
            out=w3raw_a[:], in_=_ap(self.w3, 0, [(9 * CH, 128), (1, 9 * CH)])
        )
        w3raw_b = stage.tile([CHI, 9 * CH], F32)
        nc.sync.dma_start(
            out=w3raw_b[:],
            in_=_ap(self.w3, 128 * 9 * CH, [(9 * CH, CHI), (1, 9 * CH)]),
        )

        def tapv(raw, t, n):  # [n_co, ci] view of tap t
            return raw[0:n, :].rearrange("p (ci t) -> p t ci", t=9)[:, t, :]

        # All lhsT tiles are K=128-padded with ZERO rows so every matmul
        # shares the 128-row stationary config: the PE pays ~100ns whenever
        # consecutive matmuls change stationary geometry (row or column
        # count), and a K=128 matmul streams the same N columns as a K=9
        # one. Zero weight rows turn the junk in the corresponding rhs
        # partitions into exact zeros.
        # w1: lhsT rows 0-8 = taps, rows 9-127 = 0.
        pT = pwtr.tile([128, 128], F32, tag="ps3lo")
        nc.tensor.transpose(pT[0:9, 0:CH], w1raw[:], ident[0:CH, 0:CH])
        self.w1sb = consts.tile([128, CH], BF16)
        nc.vector.memset(self.w1sb[:], 0.0)
        nc.vector.tensor_copy(self.w1sb[0:9, :], pT[0:9, 0:CH])

        # Transpose each tap to PSUM base 0 (HW requires base 0); upper
        # (shifted-partner tap) halves staged then partition-shifted to
        # partitions 64-127 by one SBUF->SBUF DMA per weight tile.
        # A-pairs carry taps (du,0)+(du,1) du=0..2; B-pair (0,2)+(1,2);
        # single is tap (2,2); conv3 splits co into [0:128] / [128:169].
        self.w2p = consts.tile([2 * CH, 3 * CH], BF16)
        self.w2pB = consts.tile([2 * CH, CH], BF16)
        self.w2s = consts.tile([2 * CH, CH], BF16)
        self.w3pA_lo = consts.tile([2 * CH, 3 * CLO], BF16)
        self.w3pA_hi = consts.tile([2 * CH, 3 * CHI], BF16)
        self.w3pB_lo = consts.tile([2 * CH, CLO], BF16)
        self.w3pB_hi = consts.tile([2 * CH, CHI], BF16)
        self.w3s_lo = consts.tile([2 * CH, CLO], BF16)
        self.w3s_hi = consts.tile([2 * CH, CHI], BF16)
        nc.vector.memset(self.w2s[CH:, :], 0.0)
        nc.vector.memset(self.w3s_lo[CH:, :], 0.0)
        nc.vector.memset(self.w3s_hi[CH:, :], 0.0)
        w2pu = stage.tile([CH, 3 * CH], BF16)
        w2puB = stage.tile([CH, CH], BF16)
        w3puA_lo = stage.tile([CH, 3 * CLO], BF16)
        w3puA_hi = stage.tile([CH, 3 * CHI], BF16)
        w3puB_lo = stage.tile([CH, CLO], BF16)
        w3puB_hi = stage.tile([CH, CHI], BF16)

        def tr(dst, raw, t, n):
            pT = pwtr.tile([CH, 128], F32, tag="ps3lo")
            nc.tensor.transpose(pT[:, 0:n], tapv(raw, t, n), ident[0:n, 0:n])
            nc.vector.tensor_copy(dst, pT[:, 0:n])

        for p in range(3):
            cw = slice(p * CH, (p + 1) * CH)
            cl = slice(p * CLO, (p + 1) * CLO)
            ch = slice(p * CHI, (p + 1) * CHI)
            tr(self.w2p[0:CH, cw], w2raw, p * 3, CH)
            tr(w2pu[:, cw], w2raw, p * 3 + 1, CH)
            tr(self.w3pA_lo[0:CH, cl], w3raw_a, p * 3, 128)
            tr(self.w3pA_hi[0:CH, ch], w3raw_b, p * 3, CHI)
            tr(w3puA_lo[:, cl], w3raw_a, p * 3 + 1, 128)
            tr(w3puA_hi[:, ch], w3raw_b, p * 3 + 1, CHI)
        tr(self.w2pB[0:CH, :], w2raw, 2, CH)
        tr(w2puB[:], w2raw, 5, CH)
        tr(self.w2s[0:CH, :], w2raw, 8, CH)
        tr(self.w3pB_lo[0:CH, :], w3raw_a, 2, 128)
        tr(self.w3pB_hi[0:CH, :], w3raw_b, 2, CHI)
        tr(w3puB_lo[:], w3raw_a, 5, 128)
        tr(w3puB_hi[:], w3raw_b, 5, CHI)
        tr(self.w3s_lo[0:CH, :], w3raw_a, 8, 128)
        tr(self.w3s_hi[0:CH, :], w3raw_b, 8, CHI)
        nc.sync.dma_start(out=self.w2p[CH:, :], in_=w2pu[:])
        nc.sync.dma_start(out=self.w2pB[CH:, :], in_=w2puB[:])
        nc.sync.dma_start(out=self.w3pA_lo[CH:, :], in_=w3puA_lo[:])
        nc.sync.dma_start(out=self.w3pA_hi[CH:, :], in_=w3puA_hi[:])
        nc.sync.dma_start(out=self.w3pB_lo[CH:, :], in_=w3puB_lo[:])
        nc.sync.dma_start(out=self.w3pB_hi[CH:, :], in_=w3puB_hi[:])

        # biases replicated into partitions 64-127 for the chunk-b relus
        # (engine lanes are partition-hardwired)
        self.b1sb = consts.tile([2 * CH, 1], F32)
        nc.scalar.dma_start(out=self.b1sb[0:CH], in_=_ap(self.b1, 0, [(1, CH), (0, 1)]))
        nc.scalar.dma_start(out=self.b1sb[CH:], in_=_ap(self.b1, 0, [(1, CH), (0, 1)]))
        self.b2sb = consts.tile([2 * CH, 1], F32)
        nc.scalar.dma_start(out=self.b2sb[0:CH], in_=_ap(self.b2, 0, [(1, CH), (0, 1)]))
        nc.scalar.dma_start(out=self.b2sb[CH:], in_=_ap(self.b2, 0, [(1, CH), (0, 1)]))
        self.b3lo = consts.tile([CLO, 1], F32)
        nc.scalar.dma_start(out=self.b3lo[:], in_=_ap(self.b3, 0, [(1, CLO), (0, 1)]))
        self.b3hi = consts.tile([CHI, 1], F32)
        nc.scalar.dma_start(out=self.b3hi[:], in_=_ap(self.b3, CLO, [(1, CHI), (0, 1)]))

        # staircase-ones: stair[:, 128] = 1, else 0; column j of the view
        # stair[:, 128-j : 192-j] is all-ones -> matmul writes the partition
        # sum into PSUM row j (zeros elsewhere, harmless under accumulation)
        stair_st = stage.tile([128, 256], F32)
        nc.vector.memset(stair_st[:], 0.0)
        nc.vector.memset(stair_st[:, 128:129], 1.0)
        self.stair = consts.tile([128, 256], BF16)
        nc.vector.tensor_copy(self.stair[:], stair_st[:])
        # hi-stair variant: ones only in rows 0..40 (t_hi's live taps) so a
        # K=128 matmul over the zero-padded t_hi reduces exactly 41 rows
        self.stair_h = consts.tile([128, 256], BF16)
        nc.vector.memset(self.stair_h[:], 0.0)
        nc.vector.tensor_copy(self.stair_h[0:CHI, :], stair_st[0:CHI, :])

        stage.release()

    # ---------------- per-strip stages ----------------

    def emit_conv1(self, i0, S, first_strip=False):
        # im2col DMAs for the whole strip are issued up-front (the 6-deep
        # imc pool holds a full strip) so the PE never waits on DMA latency
        # when the deferred matmul bursts run a strip later. imc tiles are
        # K=128-padded: rows 9..127 are zeroed once (first strip touches
        # every pool slot) and w1sb's zero rows make them inert.
        nc = self.nc
        c0 = i0 + 6
        L1 = (S + 6) * W
        L2 = (S + 3) * W
        LB = L2 + 320
        h1t = self.p_h1.tile([2 * CH, (S_STRIP + 6) * W + 772], BF16, tag="h1")
        h1b = self.p_h1b.tile([2 * CH, (S_STRIP + 4) * W + 320], BF16, tag="h1b")
        nc.gpsimd.memset(h1t[0:CH, L1 : L1 + 772], 0.0)
        nc.gpsimd.memset(h1t[CH:, L1 - 1 : L1 + 771], 0.0)
        Lh = (L1 // (2 * NC_)) * NC_
        groups = list(range(0, L1, 2 * NC_))
        imcs = {}

        def emit_dmas():
            for hs in groups:
                he = min(hs + 2 * NC_, L1)
                imc = self.p_imc.tile([128, 2 * NC_], BF16, tag="imc")
                if first_strip:
                    nc.gpsimd.memset(imc[:], 0.0)
                nc.sync.dma_start(
                    out=imc[0:9, 0 : he - hs],
                    in_=_ap(self.xb, (c0 - 5) * W - 1 + hs,
                            [(W, 3), (1, 3), (1, he - hs)]),
                )
                imcs[hs] = imc

        def emit_groups(grps):
            for hs in grps:
                he = min(hs + 2 * NC_, L1)
                imc = imcs[hs]
                for cs in range(hs, he, NC_):
                    ce = min(cs + NC_, L1)
                    pt = self.ps12.tile([CH, NC_], F32, tag="ps12")
                    _mm(nc, pt[:, 0 : ce - cs], self.w1sb[:],
                        imc[:, cs - hs : ce - hs], True, True)
                    nc.scalar.activation(
                        h1t[0:CH, cs:ce], pt[:, 0 : ce - cs], AF.Relu,
                        bias=self.b1sb[0:CH],
                    )
                    dup = nc.sync
                    if ce == Lh:
                        dup.dma_start(out=h1t[CH:, 0 : Lh - 1], in_=h1t[0:CH, 1:Lh])
                        dup.dma_start(out=h1b[0:CH, 0:Lh], in_=h1t[0:CH, 0:Lh])
                        dup.dma_start(out=h1b[CH:, 0 : Lh - W], in_=h1t[0:CH, W:Lh])
                    elif ce == L1:
                        dup.dma_start(
                            out=h1t[CH:, Lh - 1 : L1 - 1], in_=h1t[0:CH, Lh:L1]
                        )
                        dup.dma_start(out=h1b[0:CH, Lh:LB], in_=h1t[0:CH, Lh:LB])
                        dup.dma_start(
                            out=h1b[CH:, Lh - W : LB], in_=h1t[0:CH, Lh : LB + W]
                        )

        # split into three bursts so conv1's scalar-relu chain (684ns vs
        # ~290ns mm) doesn't back up the in-order PE queue in one long run
        return (h1t, h1b, emit_dmas, lambda: emit_groups(groups[:2]),
                lambda: emit_groups(groups[2:4]),
                lambda: emit_groups(groups[4:]))

    def emit_xs(self, i0, S):
        # xs[(u,v), i*W + j] = x[i0+u+i, j+v]: one contiguous read per
        # tap-row u (13 partitions x (S-1)*W+244 elements) into the spaced
        # layout; cols 244..256 of each row hold neighbor-row junk that the
        # stt views never touch. Tap-row u=9 straddles the 128-tap split:
        # taps 117..127 land in xs_lo[117:128], taps 128..129 in xs_hi[0:2].
        nc = self.nc
        LS = (S - 1) * W + HO
        xs_lo = self.p_xsl.tile([CLO, S_STRIP * W], BF16, tag="xsl")
        eng_lo = nc.gpsimd if XS_GP else nc.scalar
        eng_hi = nc.gpsimd if XS_GP else nc.sync
        for u in range(9):
            eng_lo.dma_start(
                out=xs_lo[u * K : (u + 1) * K, 0:LS],
                in_=_ap(self.xb, (i0 + u) * W, [(1, K), (1, LS)]),
            )
        eng_lo.dma_start(
            out=xs_lo[117:128, 0:LS],
            in_=_ap(self.xb, (i0 + 9) * W, [(1, 11), (1, LS)]),
        )
        xs_hi = self.p_xsh.tile([CHI, S_STRIP * W], BF16, tag="xsh")
        eng_hi.dma_start(
            out=xs_hi[0:2, 0:LS],
            in_=_ap(self.xb, (i0 + 9) * W + 11, [(1, 2), (1, LS)]),
        )
        for u in range(10, 13):
            eng_hi.dma_start(
                out=xs_hi[2 + (u - 10) * K : 2 + (u - 9) * K, 0:LS],
                in_=_ap(self.xb, (i0 + u) * W, [(1, K), (1, LS)]),
            )
        return xs_lo, xs_hi

    def emit_conv2(self, i0, S, h1t, h1b):
        # Chunk parity alternates [single K=64 | pairs K=128] and
        # [pairs | single] so same-row-size groups meet across chunk
        # boundaries: one PE row-size-transition drain per chunk.
        # Also builds the conv3 tiles h2t=[h2; h2<<1] / h2b=[h2; h2<<W]
        # via dup DMAs per half-strip.
        nc = self.nc
        L2 = (S + 3) * W
        LB = S * W + 320
        h2t = self.p_h2.tile([2 * CH, (S_STRIP + 3) * W + 772], BF16, tag="h2")
        h2b = self.p_h2b.tile([2 * CH, S_STRIP * W + 320], BF16, tag="h2b")
        nc.gpsimd.memset(h2t[0:CH, L2 : L2 + 772], 0.0)
        nc.gpsimd.memset(h2t[CH:, L2 - 1 : L2 + 771], 0.0)
        Lh = (L2 // (2 * NC_)) * NC_
        for ci, cs in enumerate(range(0, L2, NC_)):
            ce = min(cs + NC_, L2)
            pt = self.ps12.tile([CH, NC_], F32, tag="ps12")

            for p in range(3):
                off = p * W + 255
                _mm(nc, pt[:, 0 : ce - cs],
                    self.w2p[:, p * CH : (p + 1) * CH],
                    h1t[:, cs + off : ce + off], p == 0, False)
            offb = 2 + 255
            _mm(nc, pt[:, 0 : ce - cs], self.w2pB[:],
                h1b[:, cs + offb : ce + offb], False, False)
            offs = 2 * W + 2 + 255
            _mm(nc, pt[:, 0 : ce - cs], self.w2s[:],
                h1t[:, cs + offs : ce + offs], False, True)
            nc.scalar.activation(
                h2t[0:CH, cs:ce], pt[:, 0 : ce - cs], AF.Relu, bias=self.b2sb[0:CH]
            )
            dup = nc.gpsimd if DUP_GP else nc.sync
            if ce == Lh:
                dup.dma_start(out=h2t[CH:, 0 : Lh - 1], in_=h2t[0:CH, 1:Lh])
                dup.dma_start(out=h2b[0:CH, 0:Lh], in_=h2t[0:CH, 0:Lh])
                dup.dma_start(out=h2b[CH:, 0 : Lh - W], in_=h2t[0:CH, W:Lh])
            elif ce == L2:
                dup.dma_start(out=h2t[CH:, Lh - 1 : L2 - 1], in_=h2t[0:CH, Lh:L2])
                dup.dma_start(out=h2b[0:CH, Lh:LB], in_=h2t[0:CH, Lh:LB])
                dup.dma_start(
                    out=h2b[CH:, Lh - W : LB], in_=h2t[0:CH, Lh : LB + W]
                )
        return h2t, h2b

    def emit_conv3_chunk(self, i0, cs, h2t, h2b, xs_lo, xs_hi, flush):
        """conv3 + stt for one 2-row chunk; staircase matmuls are deferred.

        Every matmul is K=128 (zero-padded weights for the lone single
        tap), so the only stationary-geometry changes are M: the M=128
        block (lo pairs+single, both stairs) and the M=41 block (hi
        pairs+single). Chunk parity mirrors the block order so chunk
        boundaries are transition-free: ONE geometry change per chunk."""
        nc = self.nc
        even = self.gchunk % 2 == 0
        self.gchunk += 1
        plo = self.ps3.tile([CLO, NC_], F32, tag="ps3lo")
        phi = self.ps3.tile([CHI, NC_], F32, tag="ps3hi")

        def block_lo():
            off = 2 * W + 2 + 255
            _mm(nc, plo[:], self.w3s_lo[:],
                h2t[:, cs + off : cs + NC_ + off], True, False)
            for p in range(3):
                off = p * W + 255
                _mm(nc, plo[:], self.w3pA_lo[:, p * CLO : (p + 1) * CLO],
                    h2t[:, cs + off : cs + NC_ + off], False, False)
            offb = 2 + 255
            _mm(nc, plo[:], self.w3pB_lo[:],
                h2b[:, cs + offb : cs + NC_ + offb], False, True)

        def block_hi():
            off = 2 * W + 2 + 255
            _mm(nc, phi[:], self.w3s_hi[:],
                h2t[:, cs + off : cs + NC_ + off], True, False)
            for p in range(3):
                off = p * W + 255
                _mm(nc, phi[:], self.w3pA_hi[:, p * CHI : (p + 1) * CHI],
                    h2t[:, cs + off : cs + NC_ + off], False, False)
            offb = 2 + 255
            _mm(nc, phi[:], self.w3pB_hi[:],
                h2b[:, cs + offb : cs + NC_ + offb], False, True)

        flush_hi, flush_lo = flush
        if even:
            block_lo()
            flush_lo()
            flush_hi()
            block_hi()
        else:
            block_hi()
            flush_lo()
            flush_hi()
            block_lo()

        r2 = cs // W
        jj = (i0 + r2) // 2
        # t2 = (conv3_psum + b3) * xs, straight from PSUM on the DVE; all
        # three operands live in the 256-spaced [c, (r, col)] layout.
        # t_hi is K=128-padded: rows 41..127 are zeroed once per pool slot
        # (the first 5 chunks touch all 5 slots) and stair_h's zero rows
        # make them inert.
        t2 = self.p_t2.tile([CLO, NC_], BF16, tag="t2")
        t_hi = self.p_th.tile([128, NC_], BF16, tag="t_hi")
        if self.gchunk <= 3:
            nc.gpsimd.memset(t_hi[64:128, :], 0.0)
            nc.gpsimd.memset(t_hi[32:64, :], 0.0)
        wv_lo = plo[:].rearrange("p (r c) -> p r c", c=W)[:, :, 6 : 6 + HO]
        wv_hi = phi[:].rearrange("p (r c) -> p r c", c=W)[:, :, 6 : 6 + HO]
        xv_lo = xs_lo[:, cs : cs + NC_].rearrange("p (r c) -> p r c", c=W)[:, :, 0:HO]
        xv_hi = xs_hi[:, cs : cs + NC_].rearrange("p (r c) -> p r c", c=W)[:, :, 0:HO]
        tv_lo = t2[:].rearrange("p (r c) -> p r c", c=W)[:, :, 0:HO]
        tv_hi = t_hi[0:CHI].rearrange("p (r c) -> p r c", c=W)[:, :, 0:HO]
        nc.vector.scalar_tensor_tensor(
            out=tv_lo, in0=wv_lo, scalar=self.b3lo[:], in1=xv_lo,
            op0=ALU.add, op1=ALU.mult,
        )
        nc.vector.scalar_tensor_tensor(
            out=tv_hi, in0=wv_hi, scalar=self.b3hi[:], in1=xv_hi,
            op0=ALU.add, op1=ALU.mult,
        )
        self.pend_lo.append((t2, jj))
        self.pend_hi.append((t_hi, jj))

    def _stair_mm(self, t_t, stair, jj):
        # psum_y is one [128, 488] bank accumulating all 122 output chunks;
        # the staircase lhsT is K=128 x M=128 so both stairs share the lo
        # pairs' stationary geometry exactly (no PE reconfiguration drain).
        nc = self.nc
        rhs = t_t[:].rearrange("p (r c) -> p r c", c=W)[:, :, 0:HO]
        _mm(nc, self.psum_y[:], stair[:, 128 - jj : 256 - jj],
            rhs, self.cnt == 0, self.cnt == 2 * self.NYC - 1)
        self.cnt += 1

    def flush_stair_lo(self, keep=0):
        while len(self.pend_lo) > keep:
            t2, jj = self.pend_lo.pop(0)
            self._stair_mm(t2, self.stair, jj)

    def flush_stair_hi(self, keep=0):
        while len(self.pend_hi) > keep:
            t_hi, jj = self.pend_hi.pop(0)
            self._stair_mm(t_hi, self.stair_h, jj)

    def _build_strips(self):
        nc = self.nc
        self.NYC = (HO * HO) // 488  # 122
        self.psum_y = self.psy.tile([128, 488], F32, tag="y")
        self.pend_lo = []
        self.pend_hi = []
        self.cnt = 0
        self.gchunk = 0

        strips = []
        i0 = 0
        while i0 < HO:
            strips.append((i0, min(S_STRIP, HO - i0)))
            i0 += S_STRIP

        h1t, h1b, c1d, c1a, c1b, c1c = self.emit_conv1(*strips[0],
                                                       first_strip=True)
        c1d()
        c1a()
        c1b()
        c1c()
        xs = self.emit_xs(*strips[0])
        self.keep = 2 if PIPE else 0
        flush = (lambda: self.flush_stair_hi(keep=self.keep),
                 lambda: self.flush_stair_lo(keep=self.keep))
        for si, (i0, S) in enumerate(strips):
            if si == len(strips) - 1:
                self.keep = 0
            h2t, h2b = self.emit_conv2(i0, S, h1t, h1b)
            xs_lo, xs_hi = xs
            # prefetch next strip's xs while this strip's conv3 runs
            if si + 1 < len(strips):
                xs = self.emit_xs(*strips[si + 1])
                h1n, h1bn, c1d, c1a, c1b, c1c = self.emit_conv1(*strips[si + 1])
                c1d()
            else:
                c1a = c1b = c1c = None
            for ci, cs in enumerate(range(0, S * W, NC_)):
                self.emit_conv3_chunk(
                    i0, cs, h2t, h2b, xs_lo, xs_hi, flush=flush,
                )
                # overlap next strip's conv1 with this strip's conv3 tail,
                # split into three bursts
                if PIPE and ci == 1 and c1a is not None:
                    c1a()
                    c1a = None
                if PIPE and ci == 3 and c1b is not None:
                    c1b()
                    c1b = None
                if PIPE and ci == 5 and c1c is not None:
                    c1c()
                    c1c = None
            for fn in (c1a, c1b, c1c):
                if fn is not None:
                    fn()
            if si + 1 < len(strips):
                h1t, h1b = h1n, h1bn
            self.flush_stair_hi(keep=0)
            self.flush_stair_lo(keep=0)

        ysb = self.p_y.tile([self.NYC, 488], F32, tag="y")
        nc.vector.tensor_copy(ysb[:], self.psum_y[0 : self.NYC, :])
        nc.sync.dma_start(
            out=_ap(self.y, 0, [(488, self.NYC), (1, 488)]), in_=ysb[:]
        )


_NC_CACHE = {}


def _get_nc():
    if "nc" not in _NC_CACHE:
        _NC_CACHE["nc"] = build_nc()
    return _NC_CACHE["nc"]


def _in_maps(inputs):
    x = np.ascontiguousarray(np.asarray(inputs["x"], dtype=np.float32))
    names = ["W1", "b1", "W2", "b2", "W3", "b3"]
    ws = {n: np.ascontiguousarray(np.asarray(inputs[n], np.float32)) for n in names}
    maps = []
    for i in range(8):
        m = {"x": x[i : i + 1]}
        m.update(ws)
        maps.append(m)
    return maps


def kernel(**inputs):
    nc = _get_nc()
    res = run_bass_kernel_spmd(nc, _in_maps(inputs), list(range(8)))
    return np.concatenate([res.results[i]["y"] for i in range(8)], axis=0)


def profile(**inputs):
    nc = _get_nc()
    res = run_bass_kernel_spmd(nc, _in_maps(inputs), list(range(8)), trace=True)
    return res.exec_time_ns


if __name__ == "__main__":
    rng = np.random.RandomState(0)
    ins = {
        "x": rng.randn(8, 1, H, W).astype(np.float32),
        "W1": rng.randn(CH, 1, 3, 3).astype(np.float32) * 0.1,
        "b1": np.zeros(CH, np.float32),
        "W2": rng.randn(CH, CH, 3, 3).astype(np.float32) * 0.05,
        "b2": np.zeros(CH, np.float32),
        "W3": rng.randn(C3, CH, 3, 3).astype(np.float32) * 0.05,
        "b3": np.zeros(C3, np.float32),
    }
    print(kernel(**ins).shape)


# revision 39
# speedup vs baseline: 1.1011x; 1.1011x over previous
"""NlmCNN (weight-predicting CNN + per-pixel 13x13 weighted sum) on 8 trn2 cores.

Sharding: data-parallel over batch (8 images -> 8 cores), weights replicated.

Per-core layout trick: output y is the conv stack's result center-cropped by
6 pixels, and the receptive field of the three 3x3 convs only reaches 3 px
out, so SAME-padding never materializes: every layer is computed VALID-style
on an unpadded 256-stride flat layout. Column-wrap junk from flat shifted
reads stays confined to the outer <=3 columns of each layer, which are
discarded by the crop.

All matmul operands are bf16 (fp32 PSUM accumulation; end-to-end absmax-rel
~4e-3 vs the 2e-2 gate). bf16 is chosen over float32r because fp32-class
LDWEIGHTS runs ~2.2ns/stationary-column with FWL disabled: M=128 weight
loads (285ns) exceed the N=512 stream time (213ns) and the PE becomes
weight-load-bound (measured 426ns/matmul cadence). bf16 enables FWL and
streams the same 1 column/cycle.

Pipeline per strip of S output rows (strips software-pipelined: conv1 of
strip i+1 is emitted during strip i's conv3 phase):
  conv1: per-2-chunk im2col [9, 1024] via one 3-dim DMA -> K=9 matmul; all
         of a strip's im2col DMAs are issued a strip ahead (the imc pool
         holds a full strip) so the PE never waits on DMA latency.
  conv2/conv3: 3x3 taps packed into K=128 pair-matmuls: taps (du,0)+(du,1)
         pair on hA=[h; h<<1] (upper 64 partitions hold h shifted +1);
         taps (0,2)+(1,2) pair on hB=[h; h<<W]; only tap (2,2) is a K=64
         single. 5 matmuls per 512-px chunk for conv2, 10 for conv3
         (out channels split [0:128] M=128 / [128:169] M=41). The shifted
         tiles are built by SBUF->SBUF dup DMAs per half-strip. Chunk
         parity alternates [single | pairs] / [pairs | single] so the PE
         sees one K-row-size transition per chunk.
  einsum: patch matrix xs[t, s] = x[pos + shift(t)] split [128 | 41] taps,
         gathered bf16 by one contiguous DMA per tap-row u (row u=9 is
         split 11/2 across the two tiles); DVE scalar_tensor_tensor
         computes t2 = (conv3_lo + b3_lo) * xs_lo and
         t_hi = (conv3_hi + b3_hi) * xs_hi straight from PSUM; the
         partition reduction is "staircase ones" matmuls (K=128 over t2
         next to the pairs, K=41 over t_hi next to the K=64 singles --
         both transition-free), accumulating 2-row chunk jj into row jj
         of a persistent PSUM tile; one copy + one DMA store the image.
"""

import numpy as np

import concourse.bacc as bacc
import concourse.bass as bass
import concourse.mybir as mybir
import concourse.tile as tile
from concourse.bass_utils import run_bass_kernel_spmd

F32 = mybir.dt.float32
BF16 = mybir.dt.bfloat16
AF = mybir.ActivationFunctionType
ALU = mybir.AluOpType

H = 256
W = 256
K = 13
HO = H - K + 1  # 244
CH = 64
C3 = K * K  # 169
CLO = 128   # conv3 out-channel group sizes
CHI = C3 - 128  # 41
S_STRIP = 16
NC_ = 512  # chunk positions (2 image rows)
import os
PIPE = os.environ.get("K_PIPE", "1") == "1"      # cross-strip sw pipelining
XS_GP = os.environ.get("K_XS_GP", "1") == "1"    # xs DMAs on gpsimd queue
DUP_GP = os.environ.get("K_DUP_GP", "1") == "1"  # dup DMAs on gpsimd queue


def _ap(t, off, dims):
    return bass.AP(t, off, [list(d) for d in dims])


def _mm(nc, out, lhsT, rhs, start, stop):
    nc.tensor.matmul(out, lhsT, rhs, start=start, stop=stop)


def build_nc():
    nc = bacc.Bacc("TRN2", target_bir_lowering=False, debug=False)

    x = nc.dram_tensor("x", [1, 1, H, W], F32, kind="ExternalInput")
    w1 = nc.dram_tensor("W1", [CH, 1, 3, 3], F32, kind="ExternalInput")
    b1 = nc.dram_tensor("b1", [CH], F32, kind="ExternalInput")
    w2 = nc.dram_tensor("W2", [CH, CH, 3, 3], F32, kind="ExternalInput")
    b2 = nc.dram_tensor("b2", [CH], F32, kind="ExternalInput")
    w3 = nc.dram_tensor("W3", [C3, CH, 3, 3], F32, kind="ExternalInput")
    b3 = nc.dram_tensor("b3", [C3], F32, kind="ExternalInput")
    y = nc.dram_tensor("y", [1, 1, HO, HO], F32, kind="ExternalOutput")
    xb = nc.dram_tensor("x_b", [H * W], BF16)

    with tile.TileContext(nc) as tc:
        Body(nc, tc, x, w1, b1, w2, b2, w3, b3, y, xb).build()

    nc.compile()
    return nc


class Body:
    def __init__(self, nc, tc, x, w1, b1, w2, b2, w3, b3, y, xb):
        self.nc, self.tc = nc, tc
        self.x, self.w1, self.b1, self.w2, self.b2 = x, w1, b1, w2, b2
        self.w3, self.b3, self.y, self.xb = w3, b3, y, xb

    def build(self):
        nc, tc = self.nc, self.tc
        with (
            tc.tile_pool(name="consts", bufs=1) as consts,
            tc.tile_pool(name="t2p", bufs=5) as p_t2,
            tc.tile_pool(name="thp", bufs=5) as p_th,
            tc.tile_pool(name="imc", bufs=6) as p_imc,
            tc.tile_pool(name="h1p", bufs=2) as p_h1,
            tc.tile_pool(name="h1bp", bufs=2) as p_h1b,
            tc.tile_pool(name="h2p", bufs=2) as p_h2,
            tc.tile_pool(name="h2bp", bufs=2) as p_h2b,
            tc.tile_pool(name="xsl", bufs=2) as p_xsl,
            tc.tile_pool(name="xsh", bufs=2) as p_xsh,
            tc.tile_pool(name="yout", bufs=1) as p_y,
            tc.tile_pool(name="ps12", bufs=3, space="PSUM") as ps12,
            tc.tile_pool(name="ps3", bufs=2, space="PSUM") as ps3,
            tc.tile_pool(name="psy", bufs=1, space="PSUM") as psy,
        ):
            self.consts = consts
            self.p_t2, self.p_th, self.p_imc = p_t2, p_th, p_imc
            self.p_h1, self.p_h1b = p_h1, p_h1b
            self.p_h2, self.p_h2b = p_h2, p_h2b
            self.p_xsl, self.p_xsh = p_xsl, p_xsh
            self.p_y, self.ps12, self.ps3, self.psy = p_y, ps12, ps3, psy
            self._build_consts()
            self._build_strips()

    def _build_consts(self):
        nc, tc, consts = self.nc, self.tc, self.consts
        stage = tc.alloc_tile_pool(name="stage", bufs=1)
        # weight-prep transposes borrow ps3's "ps3lo" slot (same max tile
        # size, consts-time only) so no dedicated PSUM bank is needed
        pwtr = self.ps3

        # Weights arrive [co, ci, du, dv]; matmuls need [ci, co] per tap.
        # A strided gather DMA would be 4-byte-descriptor-bound, so load
        # contiguously and transpose on the PE instead.
        from concourse.masks import make_identity

        ident = stage.tile([128, 128], F32)
        make_identity(nc, ident[:])

        # x -> bf16 copy in DRAM (conv1 im2col + xs gather source); staged
        # FIRST so strip 0's im2col/xs DMAs can start while weight prep runs
        xst = stage.tile([128, H * W // 128], F32)
        nc.sync.dma_start(
            out=xst[:], in_=_ap(self.x, 0, [(H * W // 128, 128), (1, H * W // 128)])
        )
        xsb = stage.tile([128, H * W // 128], BF16)
        nc.vector.tensor_copy(xsb[:], xst[:])
        nc.sync.dma_start(
            out=_ap(self.xb, 0, [(H * W // 128, 128), (1, H * W // 128)]), in_=xsb[:]
        )

        w1raw = stage.tile([CH, 9], F32)
        nc.sync.dma_start(out=w1raw[:], in_=_ap(self.w1, 0, [(9, CH), (1, 9)]))
        w2raw = stage.tile([CH, 9 * CH], F32)
        nc.sync.dma_start(out=w2raw[:], in_=_ap(self.w2, 0, [(9 * CH, CH), (1, 9 * CH)]))
        w3raw_a = stage.tile([128, 9 * CH], F32)
        nc.sync.dma_start(
            out=w3raw_a[:], in_=_ap(self.w3, 0, [(9 * CH, 128), (1, 9 * CH)])
        )
        w3raw_b = stage.tile([CHI, 9 * CH], F32)
        nc.sync.dma_start(
            out=w3raw_b[:],
            in_=_ap(self.w3, 128 * 9 * CH, [(9 * CH, CHI), (1, 9 * CH)]),
        )

        def tapv(raw, t, n):  # [n_co, ci] view of tap t
            return raw[0:n, :].rearrange("p (ci t) -> p t ci", t=9)[:, t, :]

        # All lhsT tiles are K=128-padded with ZERO rows so every matmul
        # shares the 128-row stationary config: the PE pays ~100ns whenever
        # consecutive matmuls change stationary geometry (row or column
        # count), and a K=128 matmul streams the same N columns as a K=9
        # one. Zero weight rows turn the junk in the corresponding rhs
        # partitions into exact zeros.
        # w1: lhsT rows 0-8 = taps, rows 9-127 = 0.
        pT = pwtr.tile([128, 128], F32, tag="ps3lo")
        nc.tensor.transpose(pT[0:9, 0:CH], w1raw[:], ident[0:CH, 0:CH])
        self.w1sb = consts.tile([128, CH], BF16)
        nc.vector.memset(self.w1sb[:], 0.0)
        nc.vector.tensor_copy(self.w1sb[0:9, :], pT[0:9, 0:CH])

        # Transpose each tap to PSUM base 0 (HW requires base 0); upper
        # (shifted-partner tap) halves staged then partition-shifted to
        # partitions 64-127 by one SBUF->SBUF DMA per weight tile.
        # A-pairs carry taps (du,0)+(du,1) du=0..2; B-pair (0,2)+(1,2);
        # single is tap (2,2); conv3 splits co into [0:128] / [128:169].
        self.w2p = consts.tile([2 * CH, 3 * CH], BF16)
        self.w2pB = consts.tile([2 * CH, CH], BF16)
        self.w2s = consts.tile([2 * CH, CH], BF16)
        self.w3pA_lo = consts.tile([2 * CH, 3 * CLO], BF16)
        self.w3pA_hi = consts.tile([2 * CH, 3 * CHI], BF16)
        self.w3pB_lo = consts.tile([2 * CH, CLO], BF16)
        self.w3pB_hi = consts.tile([2 * CH, CHI], BF16)
        self.w3s_lo = consts.tile([2 * CH, CLO], BF16)
        self.w3s_hi = consts.tile([2 * CH, CHI], BF16)
        nc.vector.memset(self.w2s[CH:, :], 0.0)
        nc.vector.memset(self.w3s_lo[CH:, :], 0.0)
        nc.vector.memset(self.w3s_hi[CH:, :], 0.0)
        w2pu = stage.tile([CH, 3 * CH], BF16)
        w2puB = stage.tile([CH, CH], BF16)
        w3puA_lo = stage.tile([CH, 3 * CLO], BF16)
        w3puA_hi = stage.tile([CH, 3 * CHI], BF16)
        w3puB_lo = stage.tile([CH, CLO], BF16)
        w3puB_hi = stage.tile([CH, CHI], BF16)

        def tr(dst, raw, t, n):
            pT = pwtr.tile([CH, 128], F32, tag="ps3lo")
            nc.tensor.transpose(pT[:, 0:n], tapv(raw, t, n), ident[0:n, 0:n])
            nc.vector.tensor_copy(dst, pT[:, 0:n])

        for p in range(3):
            cw = slice(p * CH, (p + 1) * CH)
            cl = slice(p * CLO, (p + 1) * CLO)
            ch = slice(p * CHI, (p + 1) * CHI)
            tr(self.w2p[0:CH, cw], w2raw, p * 3, CH)
            tr(w2pu[:, cw], w2raw, p * 3 + 1, CH)
            tr(self.w3pA_lo[0:CH, cl], w3raw_a, p * 3, 128)
            tr(self.w3pA_hi[0:CH, ch], w3raw_b, p * 3, CHI)
            tr(w3puA_lo[:, cl], w3raw_a, p * 3 + 1, 128)
            tr(w3puA_hi[:, ch], w3raw_b, p * 3 + 1, CHI)
        tr(self.w2pB[0:CH, :], w2raw, 2, CH)
        tr(w2puB[:], w2raw, 5, CH)
        tr(self.w2s[0:CH, :], w2raw, 8, CH)
        tr(self.w3pB_lo[0:CH, :], w3raw_a, 2, 128)
        tr(self.w3pB_hi[0:CH, :], w3raw_b, 2, CHI)
        tr(w3puB_lo[:], w3raw_a, 5, 128)
        tr(w3puB_hi[:], w3raw_b, 5, CHI)
        tr(self.w3s_lo[0:CH, :], w3raw_a, 8, 128)
        tr(self.w3s_hi[0:CH, :], w3raw_b, 8, CHI)
        nc.sync.dma_start(out=self.w2p[CH:, :], in_=w2pu[:])
        nc.sync.dma_start(out=self.w2pB[CH:, :], in_=w2puB[:])
        nc.sync.dma_start(out=self.w3pA_lo[CH:, :], in_=w3puA_lo[:])
        nc.sync.dma_start(out=self.w3pA_hi[CH:, :], in_=w3puA_hi[:])
        nc.sync.dma_start(out=self.w3pB_lo[CH:, :], in_=w3puB_lo[:])
        nc.sync.dma_start(out=self.w3pB_hi[CH:, :], in_=w3puB_hi[:])

        # biases replicated into partitions 64-127 for the chunk-b relus
        # (engine lanes are partition-hardwired)
        self.b1sb = consts.tile([2 * CH, 1], F32)
        nc.scalar.dma_start(out=self.b1sb[0:CH], in_=_ap(self.b1, 0, [(1, CH), (0, 1)]))
        nc.scalar.dma_start(out=self.b1sb[CH:], in_=_ap(self.b1, 0, [(1, CH), (0, 1)]))
        self.b2sb = consts.tile([2 * CH, 1], F32)
        nc.scalar.dma_start(out=self.b2sb[0:CH], in_=_ap(self.b2, 0, [(1, CH), (0, 1)]))
        nc.scalar.dma_start(out=self.b2sb[CH:], in_=_ap(self.b2, 0, [(1, CH), (0, 1)]))
        self.b3lo = consts.tile([CLO, 1], F32)
        nc.scalar.dma_start(out=self.b3lo[:], in_=_ap(self.b3, 0, [(1, CLO), (0, 1)]))
        self.b3hi = consts.tile([CHI, 1], F32)
        nc.scalar.dma_start(out=self.b3hi[:], in_=_ap(self.b3, CLO, [(1, CHI), (0, 1)]))

        # staircase-ones: stair[:, 128] = 1, else 0; column j of the view
        # stair[:, 128-j : 192-j] is all-ones -> matmul writes the partition
        # sum into PSUM row j (zeros elsewhere, harmless under accumulation)
        stair_st = stage.tile([128, 256], F32)
        nc.vector.memset(stair_st[:], 0.0)
        nc.vector.memset(stair_st[:, 128:129], 1.0)
        self.stair = consts.tile([128, 256], BF16)
        nc.vector.tensor_copy(self.stair[:], stair_st[:])
        # hi-stair variant: ones only in rows 0..40 (t_hi's live taps) so a
        # K=128 matmul over the zero-padded t_hi reduces exactly 41 rows
        self.stair_h = consts.tile([128, 256], BF16)
        nc.vector.memset(self.stair_h[:], 0.0)
        nc.vector.tensor_copy(self.stair_h[0:CHI, :], stair_st[0:CHI, :])

        stage.release()

    # ---------------- per-strip stages ----------------

    def emit_conv1(self, i0, S, first_strip=False):
        # im2col DMAs for the whole strip are issued up-front (the 6-deep
        # imc pool holds a full strip) so the PE never waits on DMA latency
        # when the deferred matmul bursts run a strip later. imc tiles are
        # K=128-padded: rows 9..127 are zeroed once (first strip touches
        # every pool slot) and w1sb's zero rows make them inert.
        nc = self.nc
        c0 = i0 + 6
        L1 = (S + 6) * W
        L2 = (S + 3) * W
        LB = L2 + 320
        h1t = self.p_h1.tile([2 * CH, (S_STRIP + 6) * W + 772], BF16, tag="h1")
        h1b = self.p_h1b.tile([2 * CH, (S_STRIP + 4) * W + 320], BF16, tag="h1b")
        nc.gpsimd.memset(h1t[0:CH, L1 : L1 + 772], 0.0)
        nc.gpsimd.memset(h1t[CH:, L1 - 1 : L1 + 771], 0.0)
        Lh = (L1 // (2 * NC_)) * NC_
        groups = list(range(0, L1, 2 * NC_))
        imcs = {}

        def emit_dmas():
            for hs in groups:
                he = min(hs + 2 * NC_, L1)
                imc = self.p_imc.tile([128, 2 * NC_], BF16, tag="imc")
                if first_strip:
                    nc.gpsimd.memset(imc[:], 0.0)
                nc.sync.dma_start(
                    out=imc[0:9, 0 : he - hs],
                    in_=_ap(self.xb, (c0 - 5) * W - 1 + hs,
                            [(W, 3), (1, 3), (1, he - hs)]),
                )
                imcs[hs] = imc

        def emit_groups(grps):
            for hs in grps:
                he = min(hs + 2 * NC_, L1)
                imc = imcs[hs]
                for cs in range(hs, he, NC_):
                    ce = min(cs + NC_, L1)
                    pt = self.ps12.tile([CH, NC_], F32, tag="ps12")
                    _mm(nc, pt[:, 0 : ce - cs], self.w1sb[:],
                        imc[:, cs - hs : ce - hs], True, True)
                    nc.scalar.activation(
                        h1t[0:CH, cs:ce], pt[:, 0 : ce - cs], AF.Relu,
                        bias=self.b1sb[0:CH],
                    )
                    dup = nc.gpsimd if DUP_GP else nc.sync
                    if ce == Lh:
                        dup.dma_start(out=h1t[CH:, 0 : Lh - 1], in_=h1t[0:CH, 1:Lh])
                        dup.dma_start(out=h1b[0:CH, 0:Lh], in_=h1t[0:CH, 0:Lh])
                        dup.dma_start(out=h1b[CH:, 0 : Lh - W], in_=h1t[0:CH, W:Lh])
                    elif ce == L1:
                        dup.dma_start(
                            out=h1t[CH:, Lh - 1 : L1 - 1], in_=h1t[0:CH, Lh:L1]
                        )
                        dup.dma_start(out=h1b[0:CH, Lh:LB], in_=h1t[0:CH, Lh:LB])
                        dup.dma_start(
                            out=h1b[CH:, Lh - W : LB], in_=h1t[0:CH, Lh : LB + W]
                        )

        # split into three bursts so conv1's scalar-relu chain (684ns vs
        # ~290ns mm) doesn't back up the in-order PE queue in one long run
        return (h1t, h1b, emit_dmas, lambda: emit_groups(groups[:2]),
                lambda: emit_groups(groups[2:4]),
                lambda: emit_groups(groups[4:]))

    def emit_xs(self, i0, S):
        # xs[(u,v), i*W + j] = x[i0+u+i, j+v]: one contiguous read per
        # tap-row u (13 partitions x (S-1)*W+244 elements) into the spaced
        # layout; cols 244..256 of each row hold neighbor-row junk that the
        # stt views never touch. Tap-row u=9 straddles the 128-tap split:
        # taps 117..127 land in xs_lo[117:128], taps 128..129 in xs_hi[0:2].
        nc = self.nc
        LS = (S - 1) * W + HO
        xs_lo = self.p_xsl.tile([CLO, S_STRIP * W], BF16, tag="xsl")
        eng_lo = nc.gpsimd if XS_GP else nc.scalar
        eng_hi = nc.gpsimd if XS_GP else nc.sync
        for u in range(9):
            eng_lo.dma_start(
                out=xs_lo[u * K : (u + 1) * K, 0:LS],
                in_=_ap(self.xb, (i0 + u) * W, [(1, K), (1, LS)]),
            )
        eng_lo.dma_start(
            out=xs_lo[117:128, 0:LS],
            in_=_ap(self.xb, (i0 + 9) * W, [(1, 11), (1, LS)]),
        )
        xs_hi = self.p_xsh.tile([CHI, S_STRIP * W], BF16, tag="xsh")
        eng_hi.dma_start(
            out=xs_hi[0:2, 0:LS],
            in_=_ap(self.xb, (i0 + 9) * W + 11, [(1, 2), (1, LS)]),
        )
        for u in range(10, 13):
            eng_hi.dma_start(
                out=xs_hi[2 + (u - 10) * K : 2 + (u - 9) * K, 0:LS],
                in_=_ap(self.xb, (i0 + u) * W, [(1, K), (1, LS)]),
            )
        return xs_lo, xs_hi

    def emit_conv2(self, i0, S, h1t, h1b):
        # Chunk parity alternates [single K=64 | pairs K=128] and
        # [pairs | single] so same-row-size groups meet across chunk
        # boundaries: one PE row-size-transition drain per chunk.
        # Also builds the conv3 tiles h2t=[h2; h2<<1] / h2b=[h2; h2<<W]
        # via dup DMAs per half-strip.
        nc = self.nc
        L2 = (S + 3) * W
        LB = S * W + 320
        h2t = self.p_h2.tile([2 * CH, (S_STRIP + 3) * W + 772], BF16, tag="h2")
        h2b = self.p_h2b.tile([2 * CH, S_STRIP * W + 320], BF16, tag="h2b")
        nc.gpsimd.memset(h2t[0:CH, L2 : L2 + 772], 0.0)
        nc.gpsimd.memset(h2t[CH:, L2 - 1 : L2 + 771], 0.0)
        Lh = (L2 // (2 * NC_)) * NC_
        for ci, cs in enumerate(range(0, L2, NC_)):
            ce = min(cs + NC_, L2)
            pt = self.ps12.tile([CH, NC_], F32, tag="ps12")

            for p in range(3):
                off = p * W + 255
                _mm(nc, pt[:, 0 : ce - cs],
                    self.w2p[:, p * CH : (p + 1) * CH],
                    h1t[:, cs + off : ce + off], p == 0, False)
            offb = 2 + 255
            _mm(nc, pt[:, 0 : ce - cs], self.w2pB[:],
                h1b[:, cs + offb : ce + offb], False, False)
            offs = 2 * W + 2 + 255
            _mm(nc, pt[:, 0 : ce - cs], self.w2s[:],
                h1t[:, cs + offs : ce + offs], False, True)
            nc.scalar.activation(
                h2t[0:CH, cs:ce], pt[:, 0 : ce - cs], AF.Relu, bias=self.b2sb[0:CH]
            )
            dup = nc.gpsimd if DUP_GP else nc.sync
            if ce == Lh:
                dup.dma_start(out=h2t[CH:, 0 : Lh - 1], in_=h2t[0:CH, 1:Lh])
                dup.dma_start(out=h2b[0:CH, 0:Lh], in_=h2t[0:CH, 0:Lh])
                dup.dma_start(out=h2b[CH:, 0 : Lh - W], in_=h2t[0:CH, W:Lh])
            elif ce == L2:
                dup.dma_start(out=h2t[CH:, Lh - 1 : L2 - 1], in_=h2t[0:CH, Lh:L2])
                dup.dma_start(out=h2b[0:CH, Lh:LB], in_=h2t[0:CH, Lh:LB])
                dup.dma_start(
                    out=h2b[CH:, Lh - W : LB], in_=h2t[0:CH, Lh : LB + W]
                )
        return h2t, h2b

    def emit_conv3_chunk(self, i0, cs, h2t, h2b, xs_lo, xs_hi, flush):
        """conv3 + stt for one 2-row chunk; staircase matmuls are deferred.

        Every matmul is K=128 (zero-padded weights for the lone single
        tap), so the only stationary-geometry changes are M: the M=128
        block (lo pairs+single, both stairs) and the M=41 block (hi
        pairs+single). Chunk parity mirrors the block order so chunk
        boundaries are transition-free: ONE geometry change per chunk."""
        nc = self.nc
        even = self.gchunk % 2 == 0
        self.gchunk += 1
        plo = self.ps3.tile([CLO, NC_], F32, tag="ps3lo")
        phi = self.ps3.tile([CHI, NC_], F32, tag="ps3hi")

        def block_lo():
            off = 2 * W + 2 + 255
            _mm(nc, plo[:], self.w3s_lo[:],
                h2t[:, cs + off : cs + NC_ + off], True, False)
            for p in range(3):
                off = p * W + 255
                _mm(nc, plo[:], self.w3pA_lo[:, p * CLO : (p + 1) * CLO],
                    h2t[:, cs + off : cs + NC_ + off], False, False)
            offb = 2 + 255
            _mm(nc, plo[:], self.w3pB_lo[:],
                h2b[:, cs + offb : cs + NC_ + offb], False, True)

        def block_hi():
            off = 2 * W + 2 + 255
            _mm(nc, phi[:], self.w3s_hi[:],
                h2t[:, cs + off : cs + NC_ + off], True, False)
            for p in range(3):
                off = p * W + 255
                _mm(nc, phi[:], self.w3pA_hi[:, p * CHI : (p + 1) * CHI],
                    h2t[:, cs + off : cs + NC_ + off], False, False)
            offb = 2 + 255
            _mm(nc, phi[:], self.w3pB_hi[:],
                h2b[:, cs + offb : cs + NC_ + offb], False, True)

        flush_hi, flush_lo = flush
        if even:
            block_lo()
            flush_lo()
            flush_hi()
            block_hi()
        else:
            block_hi()
            flush_lo()
            flush_hi()
            block_lo()

        r2 = cs // W
        jj = (i0 + r2) // 2
        # t2 = (conv3_psum + b3) * xs, straight from PSUM on the DVE; all
        # three operands live in the 256-spaced [c, (r, col)] layout.
        # t_hi is K=128-padded: rows 41..127 are zeroed once per pool slot
        # (the first 5 chunks touch all 5 slots) and stair_h's zero rows
        # make them inert.
        t2 = self.p_t2.tile([CLO, NC_], BF16, tag="t2")
        t_hi = self.p_th.tile([128, NC_], BF16, tag="t_hi")
        if self.gchunk <= 5:
            nc.gpsimd.memset(t_hi[64:128, :], 0.0)
            nc.gpsimd.memset(t_hi[32:64, :], 0.0)
        wv_lo = plo[:].rearrange("p (r c) -> p r c", c=W)[:, :, 6 : 6 + HO]
        wv_hi = phi[:].rearrange("p (r c) -> p r c", c=W)[:, :, 6 : 6 + HO]
        xv_lo = xs_lo[:, cs : cs + NC_].rearrange("p (r c) -> p r c", c=W)[:, :, 0:HO]
        xv_hi = xs_hi[:, cs : cs + NC_].rearrange("p (r c) -> p r c", c=W)[:, :, 0:HO]
        tv_lo = t2[:].rearrange("p (r c) -> p r c", c=W)[:, :, 0:HO]
        tv_hi = t_hi[0:CHI].rearrange("p (r c) -> p r c", c=W)[:, :, 0:HO]
        nc.vector.scalar_tensor_tensor(
            out=tv_lo, in0=wv_lo, scalar=self.b3lo[:], in1=xv_lo,
            op0=ALU.add, op1=ALU.mult,
        )
        nc.vector.scalar_tensor_tensor(
            out=tv_hi, in0=wv_hi, scalar=self.b3hi[:], in1=xv_hi,
            op0=ALU.add, op1=ALU.mult,
        )
        self.pend_lo.append((t2, jj))
        self.pend_hi.append((t_hi, jj))

    def _stair_mm(self, t_t, stair, jj):
        # psum_y is one [128, 488] bank accumulating all 122 output chunks;
        # the staircase lhsT is K=128 x M=128 so both stairs share the lo
        # pairs' stationary geometry exactly (no PE reconfiguration drain).
        nc = self.nc
        rhs = t_t[:].rearrange("p (r c) -> p r c", c=W)[:, :, 0:HO]
        _mm(nc, self.psum_y[:], stair[:, 128 - jj : 256 - jj],
            rhs, self.cnt == 0, self.cnt == 2 * self.NYC - 1)
        self.cnt += 1

    def flush_stair_lo(self, keep=0):
        while len(self.pend_lo) > keep:
            t2, jj = self.pend_lo.pop(0)
            self._stair_mm(t2, self.stair, jj)

    def flush_stair_hi(self, keep=0):
        while len(self.pend_hi) > keep:
            t_hi, jj = self.pend_hi.pop(0)
            self._stair_mm(t_hi, self.stair_h, jj)

    def _build_strips(self):
        nc = self.nc
        self.NYC = (HO * HO) // 488  # 122
        self.psum_y = self.psy.tile([128, 488], F32, tag="y")
        self.pend_lo = []
        self.pend_hi = []
        self.cnt = 0
        self.gchunk = 0

        strips = []
        i0 = 0
        while i0 < HO:
            strips.append((i0, min(S_STRIP, HO - i0)))
            i0 += S_STRIP

        h1t, h1b, c1d, c1a, c1b, c1c = self.emit_conv1(*strips[0],
                                                       first_strip=True)
        c1d()
        c1a()
        c1b()
        c1c()
        xs = self.emit_xs(*strips[0])
        self.keep = 4 if PIPE else 0
        flush = (lambda: self.flush_stair_hi(keep=self.keep),
                 lambda: self.flush_stair_lo(keep=self.keep))
        for si, (i0, S) in enumerate(strips):
            if si == len(strips) - 1:
                self.keep = 0
            h2t, h2b = self.emit_conv2(i0, S, h1t, h1b)
            xs_lo, xs_hi = xs
            # prefetch next strip's xs while this strip's conv3 runs
            if si + 1 < len(strips):
                xs = self.emit_xs(*strips[si + 1])
                h1n, h1bn, c1d, c1a, c1b, c1c = self.emit_conv1(*strips[si + 1])
                c1d()
            else:
                c1a = c1b = c1c = None
            for ci, cs in enumerate(range(0, S * W, NC_)):
                self.emit_conv3_chunk(
                    i0, cs, h2t, h2b, xs_lo, xs_hi, flush=flush,
                )
                # overlap next strip's conv1 with this strip's conv3 tail,
                # split into three bursts
                if PIPE and ci == 1 and c1a is not None:
                    c1a()
                    c1a = None
                if PIPE and ci == 3 and c1b is not None:
                    c1b()
                    c1b = None
                if PIPE and ci == 5 and c1c is not None:
                    c1c()
                    c1c = None
            for fn in (c1a, c1b, c1c):
                if fn is not None:
                    fn()
            if si + 1 < len(strips):
                h1t, h1b = h1n, h1bn
            self.flush_stair_hi(keep=0)
            self.flush_stair_lo(keep=0)

        ysb = self.p_y.tile([self.NYC, 488], F32, tag="y")
        nc.vector.tensor_copy(ysb[:], self.psum_y[0 : self.NYC, :])
        nc.sync.dma_start(
            out=_ap(self.y, 0, [(488, self.NYC), (1, 488)]), in_=ysb[:]
        )


_NC_CACHE = {}


def _get_nc():
    if "nc" not in _NC_CACHE:
        _NC_CACHE["nc"] = build_nc()
    return _NC_CACHE["nc"]


def _in_maps(inputs):
    x = np.ascontiguousarray(np.asarray(inputs["x"], dtype=np.float32))
    names = ["W1", "b1", "W2", "b2", "W3", "b3"]
    ws = {n: np.ascontiguousarray(np.asarray(inputs[n], np.float32)) for n in names}
    maps = []
    for i in range(8):
        m = {"x": x[i : i + 1]}
        m.update(ws)
        maps.append(m)
    return maps


def kernel(**inputs):
    nc = _get_nc()
    res = run_bass_kernel_spmd(nc, _in_maps(inputs), list(range(8)))
    return np.concatenate([res.results[i]["y"] for i in range(8)], axis=0)


def profile(**inputs):
    nc = _get_nc()
    res = run_bass_kernel_spmd(nc, _in_maps(inputs), list(range(8)), trace=True)
    return res.exec_time_ns


if __name__ == "__main__":
    rng = np.random.RandomState(0)
    ins = {
        "x": rng.randn(8, 1, H, W).astype(np.float32),
        "W1": rng.randn(CH, 1, 3, 3).astype(np.float32) * 0.1,
        "b1": np.zeros(CH, np.float32),
        "W2": rng.randn(CH, CH, 3, 3).astype(np.float32) * 0.05,
        "b2": np.zeros(CH, np.float32),
        "W3": rng.randn(C3, CH, 3, 3).astype(np.float32) * 0.05,
        "b3": np.zeros(C3, np.float32),
    }
    print(kernel(**ins).shape)


# revision 40
# speedup vs baseline: 1.1584x; 1.0520x over previous
"""NlmCNN (weight-predicting CNN + per-pixel 13x13 weighted sum) on 8 trn2 cores.

Sharding: data-parallel over batch (8 images -> 8 cores), weights replicated.

Per-core layout trick: output y is the conv stack's result center-cropped by
6 pixels, and the receptive field of the three 3x3 convs only reaches 3 px
out, so SAME-padding never materializes: every layer is computed VALID-style
on an unpadded 256-stride flat layout. Column-wrap junk from flat shifted
reads stays confined to the outer <=3 columns of each layer, which are
discarded by the crop.

All matmul operands are bf16 (fp32 PSUM accumulation; end-to-end absmax-rel
~4e-3 vs the 2e-2 gate). bf16 is chosen over float32r because fp32-class
LDWEIGHTS runs ~2.2ns/stationary-column with FWL disabled: M=128 weight
loads (285ns) exceed the N=512 stream time (213ns) and the PE becomes
weight-load-bound (measured 426ns/matmul cadence). bf16 enables FWL and
streams the same 1 column/cycle.

Pipeline per strip of S output rows (strips software-pipelined: conv1 of
strip i+1 is emitted during strip i's conv3 phase):
  conv1: per-2-chunk im2col [9, 1024] via one 3-dim DMA -> K=9 matmul; all
         of a strip's im2col DMAs are issued a strip ahead (the imc pool
         holds a full strip) so the PE never waits on DMA latency.
  conv2/conv3: 3x3 taps packed into K=128 pair-matmuls: taps (du,0)+(du,1)
         pair on hA=[h; h<<1] (upper 64 partitions hold h shifted +1);
         taps (0,2)+(1,2) pair on hB=[h; h<<W]; only tap (2,2) is a K=64
         single. 5 matmuls per 512-px chunk for conv2, 10 for conv3
         (out channels split [0:128] M=128 / [128:169] M=41). The shifted
         tiles are built by SBUF->SBUF dup DMAs per half-strip. Chunk
         parity alternates [single | pairs] / [pairs | single] so the PE
         sees one K-row-size transition per chunk.
  einsum: patch matrix xs[t, s] = x[pos + shift(t)] split [128 | 41] taps,
         gathered bf16 by one contiguous DMA per tap-row u (row u=9 is
         split 11/2 across the two tiles); DVE scalar_tensor_tensor
         computes t2 = (conv3_lo + b3_lo) * xs_lo and
         t_hi = (conv3_hi + b3_hi) * xs_hi straight from PSUM; the
         partition reduction is "staircase ones" matmuls (K=128 over t2
         next to the pairs, K=41 over t_hi next to the K=64 singles --
         both transition-free), accumulating 2-row chunk jj into row jj
         of a persistent PSUM tile; one copy + one DMA store the image.
"""

import numpy as np

import concourse.bacc as bacc
import concourse.bass as bass
import concourse.mybir as mybir
import concourse.tile as tile
from concourse.bass_utils import run_bass_kernel_spmd

F32 = mybir.dt.float32
BF16 = mybir.dt.bfloat16
AF = mybir.ActivationFunctionType
ALU = mybir.AluOpType

H = 256
W = 256
K = 13
HO = H - K + 1  # 244
CH = 64
C3 = K * K  # 169
CLO = 128   # conv3 out-channel group sizes
CHI = C3 - 128  # 41
S_STRIP = 16
NC_ = 512  # chunk positions (2 image rows)
import os
PIPE = os.environ.get("K_PIPE", "1") == "1"      # cross-strip sw pipelining
XS_GP = os.environ.get("K_XS_GP", "1") == "1"    # xs DMAs on gpsimd queue
DUP_GP = os.environ.get("K_DUP_GP", "1") == "1"  # dup DMAs on gpsimd queue


def _ap(t, off, dims):
    return bass.AP(t, off, [list(d) for d in dims])


def _mm(nc, out, lhsT, rhs, start, stop):
    nc.tensor.matmul(out, lhsT, rhs, start=start, stop=stop)


def build_nc():
    nc = bacc.Bacc("TRN2", target_bir_lowering=False, debug=False)

    x = nc.dram_tensor("x", [1, 1, H, W], F32, kind="ExternalInput")
    w1 = nc.dram_tensor("W1", [CH, 1, 3, 3], F32, kind="ExternalInput")
    b1 = nc.dram_tensor("b1", [CH], F32, kind="ExternalInput")
    w2 = nc.dram_tensor("W2", [CH, CH, 3, 3], F32, kind="ExternalInput")
    b2 = nc.dram_tensor("b2", [CH], F32, kind="ExternalInput")
    w3 = nc.dram_tensor("W3", [C3, CH, 3, 3], F32, kind="ExternalInput")
    b3 = nc.dram_tensor("b3", [C3], F32, kind="ExternalInput")
    y = nc.dram_tensor("y", [1, 1, HO, HO], F32, kind="ExternalOutput")
    xb = nc.dram_tensor("x_b", [H * W], BF16)

    with tile.TileContext(nc) as tc:
        Body(nc, tc, x, w1, b1, w2, b2, w3, b3, y, xb).build()

    nc.compile()
    return nc


class Body:
    def __init__(self, nc, tc, x, w1, b1, w2, b2, w3, b3, y, xb):
        self.nc, self.tc = nc, tc
        self.x, self.w1, self.b1, self.w2, self.b2 = x, w1, b1, w2, b2
        self.w3, self.b3, self.y, self.xb = w3, b3, y, xb

    def build(self):
        nc, tc = self.nc, self.tc
        with (
            tc.tile_pool(name="consts", bufs=1) as consts,
            tc.tile_pool(name="t2p", bufs=5) as p_t2,
            tc.tile_pool(name="thp", bufs=5) as p_th,
            tc.tile_pool(name="imc", bufs=6) as p_imc,
            tc.tile_pool(name="h1p", bufs=2) as p_h1,
            tc.tile_pool(name="h1bp", bufs=2) as p_h1b,
            tc.tile_pool(name="h2p", bufs=2) as p_h2,
            tc.tile_pool(name="h2bp", bufs=2) as p_h2b,
            tc.tile_pool(name="xsl", bufs=2) as p_xsl,
            tc.tile_pool(name="xsh", bufs=2) as p_xsh,
            tc.tile_pool(name="yout", bufs=1) as p_y,
            tc.tile_pool(name="ps12", bufs=3, space="PSUM") as ps12,
            tc.tile_pool(name="ps3", bufs=2, space="PSUM") as ps3,
            tc.tile_pool(name="psy", bufs=1, space="PSUM") as psy,
        ):
            self.consts = consts
            self.p_t2, self.p_th, self.p_imc = p_t2, p_th, p_imc
            self.p_h1, self.p_h1b = p_h1, p_h1b
            self.p_h2, self.p_h2b = p_h2, p_h2b
            self.p_xsl, self.p_xsh = p_xsl, p_xsh
            self.p_y, self.ps12, self.ps3, self.psy = p_y, ps12, ps3, psy
            self._build_consts()
            self._build_strips()

    def _build_consts(self):
        nc, tc, consts = self.nc, self.tc, self.consts
        stage = tc.alloc_tile_pool(name="stage", bufs=1)
        # weight-prep transposes borrow ps3's "ps3lo" slot (same max tile
        # size, consts-time only) so no dedicated PSUM bank is needed
        pwtr = self.ps3

        # Weights arrive [co, ci, du, dv]; matmuls need [ci, co] per tap.
        # A strided gather DMA would be 4-byte-descriptor-bound, so load
        # contiguously and transpose on the PE instead.
        from concourse.masks import make_identity

        ident = stage.tile([128, 128], F32)
        make_identity(nc, ident[:])

        # x -> bf16 copy in DRAM (conv1 im2col + xs gather source); staged
        # FIRST so strip 0's im2col/xs DMAs can start while weight prep runs
        xst = stage.tile([128, H * W // 128], F32)
        nc.sync.dma_start(
            out=xst[:], in_=_ap(self.x, 0, [(H * W // 128, 128), (1, H * W // 128)])
        )
        xsb = stage.tile([128, H * W // 128], BF16)
        nc.vector.tensor_copy(xsb[:], xst[:])
        nc.sync.dma_start(
            out=_ap(self.xb, 0, [(H * W // 128, 128), (1, H * W // 128)]), in_=xsb[:]
        )

        w1raw = stage.tile([CH, 9], F32)
        nc.sync.dma_start(out=w1raw[:], in_=_ap(self.w1, 0, [(9, CH), (1, 9)]))
        w2raw = stage.tile([CH, 9 * CH], F32)
        nc.sync.dma_start(out=w2raw[:], in_=_ap(self.w2, 0, [(9 * CH, CH), (1, 9 * CH)]))
        w3raw_a = stage.tile([128, 9 * CH], F32)
        nc.sync.dma_start(
            out=w3raw_a[:], in_=_ap(self.w3, 0, [(9 * CH, 128), (1, 9 * CH)])
        )
        w3raw_b = stage.tile([CHI, 9 * CH], F32)
        nc.sync.dma_start(
            out=w3raw_b[:],
            in_=_ap(self.w3, 128 * 9 * CH, [(9 * CH, CHI), (1, 9 * CH)]),
        )

        def tapv(raw, t, n):  # [n_co, ci] view of tap t
            return raw[0:n, :].rearrange("p (ci t) -> p t ci", t=9)[:, t, :]

        # All lhsT tiles are K=128-padded with ZERO rows so every matmul
        # shares the 128-row stationary config: the PE pays ~100ns whenever
        # consecutive matmuls change stationary geometry (row or column
        # count), and a K=128 matmul streams the same N columns as a K=9
        # one. Zero weight rows turn the junk in the corresponding rhs
        # partitions into exact zeros.
        # w1: lhsT rows 0-8 = taps, rows 9-127 = 0.
        pT = pwtr.tile([128, 128], F32, tag="ps3lo")
        nc.tensor.transpose(pT[0:9, 0:CH], w1raw[:], ident[0:CH, 0:CH])
        self.w1sb = consts.tile([128, 128], BF16)
        nc.vector.memset(self.w1sb[:], 0.0)
        nc.vector.tensor_copy(self.w1sb[0:9, 0:CH], pT[0:9, 0:CH])

        # Transpose each tap to PSUM base 0 (HW requires base 0); upper
        # (shifted-partner tap) halves staged then partition-shifted to
        # partitions 64-127 by one SBUF->SBUF DMA per weight tile.
        # A-pairs carry taps (du,0)+(du,1) du=0..2; B-pair (0,2)+(1,2);
        # single is tap (2,2); conv3 splits co into [0:128] / [128:169].
        # every lhsT is [128 x 128] (zero row/col padding): uniform
        # stationary geometry means the PE never pays a reconfiguration
        # drain, and NumWeights==128 keeps FWL on for every load
        self.w2p = consts.tile([128, 3 * 128], BF16)
        self.w2pB = consts.tile([128, 128], BF16)
        self.w2s = consts.tile([128, 128], BF16)
        self.w3pA_lo = consts.tile([128, 3 * CLO], BF16)
        self.w3pA_hi = consts.tile([128, 3 * 128], BF16)
        self.w3pB_lo = consts.tile([128, CLO], BF16)
        self.w3pB_hi = consts.tile([128, 128], BF16)
        self.w3s_lo = consts.tile([128, CLO], BF16)
        self.w3s_hi = consts.tile([128, 128], BF16)
        for wt in (self.w2p, self.w2pB, self.w2s, self.w3pA_hi,
                   self.w3pB_hi, self.w3s_lo, self.w3s_hi):
            nc.vector.memset(wt[:], 0.0)
        w2pu = stage.tile([CH, 3 * CH], BF16)
        w2puB = stage.tile([CH, CH], BF16)
        w3puA_lo = stage.tile([CH, 3 * CLO], BF16)
        w3puA_hi = stage.tile([CH, 3 * CHI], BF16)
        w3puB_lo = stage.tile([CH, CLO], BF16)
        w3puB_hi = stage.tile([CH, CHI], BF16)

        def tr(dst, raw, t, n):
            pT = pwtr.tile([CH, 128], F32, tag="ps3lo")
            nc.tensor.transpose(pT[:, 0:n], tapv(raw, t, n), ident[0:n, 0:n])
            nc.vector.tensor_copy(dst, pT[:, 0:n])

        for p in range(3):
            cw = slice(p * CH, (p + 1) * CH)
            cl = slice(p * CLO, (p + 1) * CLO)
            ch = slice(p * CHI, (p + 1) * CHI)
            cw2 = slice(p * 128, p * 128 + CH)
            ch2 = slice(p * 128, p * 128 + CHI)
            tr(self.w2p[0:CH, cw2], w2raw, p * 3, CH)
            tr(w2pu[:, cw], w2raw, p * 3 + 1, CH)
            tr(self.w3pA_lo[0:CH, cl], w3raw_a, p * 3, 128)
            tr(self.w3pA_hi[0:CH, ch2], w3raw_b, p * 3, CHI)
            tr(w3puA_lo[:, cl], w3raw_a, p * 3 + 1, 128)
            tr(w3puA_hi[:, ch], w3raw_b, p * 3 + 1, CHI)
        tr(self.w2pB[0:CH, 0:CH], w2raw, 2, CH)
        tr(w2puB[:], w2raw, 5, CH)
        tr(self.w2s[0:CH, 0:CH], w2raw, 8, CH)
        tr(self.w3pB_lo[0:CH, :], w3raw_a, 2, 128)
        tr(self.w3pB_hi[0:CH, 0:CHI], w3raw_b, 2, CHI)
        tr(w3puB_lo[:], w3raw_a, 5, 128)
        tr(w3puB_hi[:], w3raw_b, 5, CHI)
        tr(self.w3s_lo[0:CH, :], w3raw_a, 8, 128)
        tr(self.w3s_hi[0:CH, 0:CHI], w3raw_b, 8, CHI)
        for p in range(3):
            nc.sync.dma_start(
                out=self.w2p[CH:, p * 128 : p * 128 + CH],
                in_=w2pu[:, p * CH : (p + 1) * CH])
            nc.sync.dma_start(
                out=self.w3pA_hi[CH:, p * 128 : p * 128 + CHI],
                in_=w3puA_hi[:, p * CHI : (p + 1) * CHI])
        nc.sync.dma_start(out=self.w2pB[CH:, 0:CH], in_=w2puB[:])
        nc.sync.dma_start(out=self.w3pA_lo[CH:, :], in_=w3puA_lo[:])
        nc.sync.dma_start(out=self.w3pB_lo[CH:, :], in_=w3puB_lo[:])
        nc.sync.dma_start(out=self.w3pB_hi[CH:, 0:CHI], in_=w3puB_hi[:])

        # biases replicated into partitions 64-127 for the chunk-b relus
        # (engine lanes are partition-hardwired)
        self.b1sb = consts.tile([2 * CH, 1], F32)
        nc.scalar.dma_start(out=self.b1sb[0:CH], in_=_ap(self.b1, 0, [(1, CH), (0, 1)]))
        nc.scalar.dma_start(out=self.b1sb[CH:], in_=_ap(self.b1, 0, [(1, CH), (0, 1)]))
        self.b2sb = consts.tile([2 * CH, 1], F32)
        nc.scalar.dma_start(out=self.b2sb[0:CH], in_=_ap(self.b2, 0, [(1, CH), (0, 1)]))
        nc.scalar.dma_start(out=self.b2sb[CH:], in_=_ap(self.b2, 0, [(1, CH), (0, 1)]))
        self.b3lo = consts.tile([CLO, 1], F32)
        nc.scalar.dma_start(out=self.b3lo[:], in_=_ap(self.b3, 0, [(1, CLO), (0, 1)]))
        self.b3hi = consts.tile([CHI, 1], F32)
        nc.scalar.dma_start(out=self.b3hi[:], in_=_ap(self.b3, CLO, [(1, CHI), (0, 1)]))

        # staircase-ones: stair[:, 128] = 1, else 0; column j of the view
        # stair[:, 128-j : 192-j] is all-ones -> matmul writes the partition
        # sum into PSUM row j (zeros elsewhere, harmless under accumulation)
        stair_st = stage.tile([128, 256], F32)
        nc.vector.memset(stair_st[:], 0.0)
        nc.vector.memset(stair_st[:, 128:129], 1.0)
        self.stair = consts.tile([128, 256], BF16)
        nc.vector.tensor_copy(self.stair[:], stair_st[:])
        # hi-stair variant: ones only in rows 0..40 (t_hi's live taps) so a
        # K=128 matmul over the zero-padded t_hi reduces exactly 41 rows
        self.stair_h = consts.tile([128, 256], BF16)
        nc.vector.memset(self.stair_h[:], 0.0)
        nc.vector.tensor_copy(self.stair_h[0:CHI, :], stair_st[0:CHI, :])

        stage.release()

    # ---------------- per-strip stages ----------------

    def emit_conv1(self, i0, S, first_strip=False):
        # im2col DMAs for the whole strip are issued up-front (the 6-deep
        # imc pool holds a full strip) so the PE never waits on DMA latency
        # when the deferred matmul bursts run a strip later. imc tiles are
        # K=128-padded: rows 9..127 are zeroed once (first strip touches
        # every pool slot) and w1sb's zero rows make them inert.
        nc = self.nc
        c0 = i0 + 6
        L1 = (S + 6) * W
        L2 = (S + 3) * W
        LB = L2 + 320
        h1t = self.p_h1.tile([2 * CH, (S_STRIP + 6) * W + 772], BF16, tag="h1")
        h1b = self.p_h1b.tile([2 * CH, (S_STRIP + 4) * W + 320], BF16, tag="h1b")
        nc.gpsimd.memset(h1t[0:CH, L1 : L1 + 772], 0.0)
        nc.gpsimd.memset(h1t[CH:, L1 - 1 : L1 + 771], 0.0)
        Lh = (L1 // (2 * NC_)) * NC_
        groups = list(range(0, L1, 2 * NC_))
        imcs = {}

        def emit_dmas():
            for hs in groups:
                he = min(hs + 2 * NC_, L1)
                imc = self.p_imc.tile([128, 2 * NC_], BF16, tag="imc")
                if first_strip:
                    nc.gpsimd.memset(imc[:], 0.0)
                nc.sync.dma_start(
                    out=imc[0:9, 0 : he - hs],
                    in_=_ap(self.xb, (c0 - 5) * W - 1 + hs,
                            [(W, 3), (1, 3), (1, he - hs)]),
                )
                imcs[hs] = imc

        def emit_groups(grps):
            for hs in grps:
                he = min(hs + 2 * NC_, L1)
                imc = imcs[hs]
                for cs in range(hs, he, NC_):
                    ce = min(cs + NC_, L1)
                    pt = self.ps12.tile([128, NC_], F32, tag="ps12")
                    _mm(nc, pt[:, 0 : ce - cs], self.w1sb[:],
                        imc[:, cs - hs : ce - hs], True, True)
                    nc.scalar.activation(
                        h1t[0:CH, cs:ce], pt[0:CH, 0 : ce - cs], AF.Relu,
                        bias=self.b1sb[0:CH],
                    )
                    dup = nc.gpsimd if DUP_GP else nc.sync
                    if ce == Lh:
                        dup.dma_start(out=h1t[CH:, 0 : Lh - 1], in_=h1t[0:CH, 1:Lh])
                        dup.dma_start(out=h1b[0:CH, 0:Lh], in_=h1t[0:CH, 0:Lh])
                        dup.dma_start(out=h1b[CH:, 0 : Lh - W], in_=h1t[0:CH, W:Lh])
                    elif ce == L1:
                        dup.dma_start(
                            out=h1t[CH:, Lh - 1 : L1 - 1], in_=h1t[0:CH, Lh:L1]
                        )
                        dup.dma_start(out=h1b[0:CH, Lh:LB], in_=h1t[0:CH, Lh:LB])
                        dup.dma_start(
                            out=h1b[CH:, Lh - W : LB], in_=h1t[0:CH, Lh : LB + W]
                        )

        # split into three bursts so conv1's scalar-relu chain (684ns vs
        # ~290ns mm) doesn't back up the in-order PE queue in one long run
        return (h1t, h1b, emit_dmas, lambda: emit_groups(groups[:2]),
                lambda: emit_groups(groups[2:4]),
                lambda: emit_groups(groups[4:]))

    def emit_xs(self, i0, S):
        # xs[(u,v), i*W + j] = x[i0+u+i, j+v]: one contiguous read per
        # tap-row u (13 partitions x (S-1)*W+244 elements) into the spaced
        # layout; cols 244..256 of each row hold neighbor-row junk that the
        # stt views never touch. Tap-row u=9 straddles the 128-tap split:
        # taps 117..127 land in xs_lo[117:128], taps 128..129 in xs_hi[0:2].
        nc = self.nc
        LS = (S - 1) * W + HO
        xs_lo = self.p_xsl.tile([CLO, S_STRIP * W], BF16, tag="xsl")
        eng_lo = nc.gpsimd if XS_GP else nc.scalar
        eng_hi = nc.gpsimd if XS_GP else nc.sync
        for u in range(9):
            eng_lo.dma_start(
                out=xs_lo[u * K : (u + 1) * K, 0:LS],
                in_=_ap(self.xb, (i0 + u) * W, [(1, K), (1, LS)]),
            )
        eng_lo.dma_start(
            out=xs_lo[117:128, 0:LS],
            in_=_ap(self.xb, (i0 + 9) * W, [(1, 11), (1, LS)]),
        )
        xs_hi = self.p_xsh.tile([CHI, S_STRIP * W], BF16, tag="xsh")
        eng_hi.dma_start(
            out=xs_hi[0:2, 0:LS],
            in_=_ap(self.xb, (i0 + 9) * W + 11, [(1, 2), (1, LS)]),
        )
        for u in range(10, 13):
            eng_hi.dma_start(
                out=xs_hi[2 + (u - 10) * K : 2 + (u - 9) * K, 0:LS],
                in_=_ap(self.xb, (i0 + u) * W, [(1, K), (1, LS)]),
            )
        return xs_lo, xs_hi

    def emit_conv2(self, i0, S, h1t, h1b):
        # Chunk parity alternates [single K=64 | pairs K=128] and
        # [pairs | single] so same-row-size groups meet across chunk
        # boundaries: one PE row-size-transition drain per chunk.
        # Also builds the conv3 tiles h2t=[h2; h2<<1] / h2b=[h2; h2<<W]
        # via dup DMAs per half-strip.
        nc = self.nc
        L2 = (S + 3) * W
        LB = S * W + 320
        h2t = self.p_h2.tile([2 * CH, (S_STRIP + 3) * W + 772], BF16, tag="h2")
        h2b = self.p_h2b.tile([2 * CH, S_STRIP * W + 320], BF16, tag="h2b")
        nc.gpsimd.memset(h2t[0:CH, L2 : L2 + 772], 0.0)
        nc.gpsimd.memset(h2t[CH:, L2 - 1 : L2 + 771], 0.0)
        Lh = (L2 // (2 * NC_)) * NC_
        for ci, cs in enumerate(range(0, L2, NC_)):
            ce = min(cs + NC_, L2)
            pt = self.ps12.tile([128, NC_], F32, tag="ps12")

            for p in range(3):
                off = p * W + 255
                _mm(nc, pt[:, 0 : ce - cs],
                    self.w2p[:, p * 128 : (p + 1) * 128],
                    h1t[:, cs + off : ce + off], p == 0, False)
            offb = 2 + 255
            _mm(nc, pt[:, 0 : ce - cs], self.w2pB[:],
                h1b[:, cs + offb : ce + offb], False, False)
            offs = 2 * W + 2 + 255
            _mm(nc, pt[:, 0 : ce - cs], self.w2s[:],
                h1t[:, cs + offs : ce + offs], False, True)
            nc.scalar.activation(
                h2t[0:CH, cs:ce], pt[0:CH, 0 : ce - cs], AF.Relu,
                bias=self.b2sb[0:CH]
            )
            dup = nc.gpsimd if DUP_GP else nc.sync
            if ce == Lh:
                dup.dma_start(out=h2t[CH:, 0 : Lh - 1], in_=h2t[0:CH, 1:Lh])
                dup.dma_start(out=h2b[0:CH, 0:Lh], in_=h2t[0:CH, 0:Lh])
                dup.dma_start(out=h2b[CH:, 0 : Lh - W], in_=h2t[0:CH, W:Lh])
            elif ce == L2:
                dup.dma_start(out=h2t[CH:, Lh - 1 : L2 - 1], in_=h2t[0:CH, Lh:L2])
                dup.dma_start(out=h2b[0:CH, Lh:LB], in_=h2t[0:CH, Lh:LB])
                dup.dma_start(
                    out=h2b[CH:, Lh - W : LB], in_=h2t[0:CH, Lh : LB + W]
                )
        return h2t, h2b

    def emit_conv3_chunk(self, i0, cs, h2t, h2b, xs_lo, xs_hi, flush):
        """conv3 + stt for one 2-row chunk; staircase matmuls are deferred.

        Every matmul is K=128 (zero-padded weights for the lone single
        tap), so the only stationary-geometry changes are M: the M=128
        block (lo pairs+single, both stairs) and the M=41 block (hi
        pairs+single). Chunk parity mirrors the block order so chunk
        boundaries are transition-free: ONE geometry change per chunk."""
        nc = self.nc
        even = self.gchunk % 2 == 0
        self.gchunk += 1
        plo = self.ps3.tile([CLO, NC_], F32, tag="ps3lo")
        phi = self.ps3.tile([128, NC_], F32, tag="ps3hi")

        def block_lo():
            off = 2 * W + 2 + 255
            _mm(nc, plo[:], self.w3s_lo[:],
                h2t[:, cs + off : cs + NC_ + off], True, False)
            for p in range(3):
                off = p * W + 255
                _mm(nc, plo[:], self.w3pA_lo[:, p * CLO : (p + 1) * CLO],
                    h2t[:, cs + off : cs + NC_ + off], False, False)
            offb = 2 + 255
            _mm(nc, plo[:], self.w3pB_lo[:],
                h2b[:, cs + offb : cs + NC_ + offb], False, True)

        def block_hi():
            off = 2 * W + 2 + 255
            _mm(nc, phi[:], self.w3s_hi[:],
                h2t[:, cs + off : cs + NC_ + off], True, False)
            for p in range(3):
                off = p * W + 255
                _mm(nc, phi[:], self.w3pA_hi[:, p * 128 : (p + 1) * 128],
                    h2t[:, cs + off : cs + NC_ + off], False, False)
            offb = 2 + 255
            _mm(nc, phi[:], self.w3pB_hi[:],
                h2b[:, cs + offb : cs + NC_ + offb], False, True)

        flush_hi, flush_lo = flush
        if even:
            block_lo()
            flush_lo()
            flush_hi()
            block_hi()
        else:
            block_hi()
            flush_lo()
            flush_hi()
            block_lo()

        r2 = cs // W
        jj = (i0 + r2) // 2
        # t2 = (conv3_psum + b3) * xs, straight from PSUM on the DVE; all
        # three operands live in the 256-spaced [c, (r, col)] layout.
        # t_hi is K=128-padded: rows 41..127 are zeroed once per pool slot
        # (the first 5 chunks touch all 5 slots) and stair_h's zero rows
        # make them inert.
        t2 = self.p_t2.tile([CLO, NC_], BF16, tag="t2")
        t_hi = self.p_th.tile([128, NC_], BF16, tag="t_hi")
        if self.gchunk <= 5:
            nc.gpsimd.memset(t_hi[64:128, :], 0.0)
            nc.gpsimd.memset(t_hi[32:64, :], 0.0)
        wv_lo = plo[:].rearrange("p (r c) -> p r c", c=W)[:, :, 6 : 6 + HO]
        wv_hi = phi[0:CHI].rearrange("p (r c) -> p r c", c=W)[:, :, 6 : 6 + HO]
        xv_lo = xs_lo[:, cs : cs + NC_].rearrange("p (r c) -> p r c", c=W)[:, :, 0:HO]
        xv_hi = xs_hi[:, cs : cs + NC_].rearrange("p (r c) -> p r c", c=W)[:, :, 0:HO]
        tv_lo = t2[:].rearrange("p (r c) -> p r c", c=W)[:, :, 0:HO]
        tv_hi = t_hi[0:CHI].rearrange("p (r c) -> p r c", c=W)[:, :, 0:HO]
        nc.vector.scalar_tensor_tensor(
            out=tv_lo, in0=wv_lo, scalar=self.b3lo[:], in1=xv_lo,
            op0=ALU.add, op1=ALU.mult,
        )
        nc.vector.scalar_tensor_tensor(
            out=tv_hi, in0=wv_hi, scalar=self.b3hi[:], in1=xv_hi,
            op0=ALU.add, op1=ALU.mult,
        )
        self.pend_lo.append((t2, jj))
        self.pend_hi.append((t_hi, jj))

    def _stair_mm(self, t_t, stair, jj):
        # psum_y is one [128, 488] bank accumulating all 122 output chunks;
        # the staircase lhsT is K=128 x M=128 so both stairs share the lo
        # pairs' stationary geometry exactly (no PE reconfiguration drain).
        nc = self.nc
        rhs = t_t[:].rearrange("p (r c) -> p r c", c=W)[:, :, 0:HO]
        _mm(nc, self.psum_y[:], stair[:, 128 - jj : 256 - jj],
            rhs, self.cnt == 0, self.cnt == 2 * self.NYC - 1)
        self.cnt += 1

    def flush_stair_lo(self, keep=0):
        while len(self.pend_lo) > keep:
            t2, jj = self.pend_lo.pop(0)
            self._stair_mm(t2, self.stair, jj)

    def flush_stair_hi(self, keep=0):
        while len(self.pend_hi) > keep:
            t_hi, jj = self.pend_hi.pop(0)
            self._stair_mm(t_hi, self.stair_h, jj)

    def _build_strips(self):
        nc = self.nc
        self.NYC = (HO * HO) // 488  # 122
        self.psum_y = self.psy.tile([128, 488], F32, tag="y")
        self.pend_lo = []
        self.pend_hi = []
        self.cnt = 0
        self.gchunk = 0

        strips = []
        i0 = 0
        while i0 < HO:
            strips.append((i0, min(S_STRIP, HO - i0)))
            i0 += S_STRIP

        h1t, h1b, c1d, c1a, c1b, c1c = self.emit_conv1(*strips[0],
                                                       first_strip=True)
        c1d()
        c1a()
        c1b()
        c1c()
        xs = self.emit_xs(*strips[0])
        self.keep = 4 if PIPE else 0
        flush = (lambda: self.flush_stair_hi(keep=self.keep),
                 lambda: self.flush_stair_lo(keep=self.keep))
        for si, (i0, S) in enumerate(strips):
            if si == len(strips) - 1:
                self.keep = 0
            h2t, h2b = self.emit_conv2(i0, S, h1t, h1b)
            xs_lo, xs_hi = xs
            # prefetch next strip's xs while this strip's conv3 runs
            if si + 1 < len(strips):
                xs = self.emit_xs(*strips[si + 1])
                h1n, h1bn, c1d, c1a, c1b, c1c = self.emit_conv1(*strips[si + 1])
                c1d()
            else:
                c1a = c1b = c1c = None
            for ci, cs in enumerate(range(0, S * W, NC_)):
                self.emit_conv3_chunk(
                    i0, cs, h2t, h2b, xs_lo, xs_hi, flush=flush,
                )
                # overlap next strip's conv1 with this strip's conv3 tail,
                # split into three bursts
                if PIPE and ci == 1 and c1a is not None:
                    c1a()
                    c1a = None
                if PIPE and ci == 3 and c1b is not None:
                    c1b()
                    c1b = None
                if PIPE and ci == 5 and c1c is not None:
                    c1c()
                    c1c = None
            for fn in (c1a, c1b, c1c):
                if fn is not None:
                    fn()
            if si + 1 < len(strips):
                h1t, h1b = h1n, h1bn
            self.flush_stair_hi(keep=0)
            self.flush_stair_lo(keep=0)

        ysb = self.p_y.tile([self.NYC, 488], F32, tag="y")
        nc.vector.tensor_copy(ysb[:], self.psum_y[0 : self.NYC, :])
        nc.sync.dma_start(
            out=_ap(self.y, 0, [(488, self.NYC), (1, 488)]), in_=ysb[:]
        )


_NC_CACHE = {}


def _get_nc():
    if "nc" not in _NC_CACHE:
        _NC_CACHE["nc"] = build_nc()
    return _NC_CACHE["nc"]


def _in_maps(inputs):
    x = np.ascontiguousarray(np.asarray(inputs["x"], dtype=np.float32))
    names = ["W1", "b1", "W2", "b2", "W3", "b3"]
    ws = {n: np.ascontiguousarray(np.asarray(inputs[n], np.float32)) for n in names}
    maps = []
    for i in range(8):
        m = {"x": x[i : i + 1]}
        m.update(ws)
        maps.append(m)
    return maps


def kernel(**inputs):
    nc = _get_nc()
    res = run_bass_kernel_spmd(nc, _in_maps(inputs), list(range(8)))
    return np.concatenate([res.results[i]["y"] for i in range(8)], axis=0)


def profile(**inputs):
    nc = _get_nc()
    res = run_bass_kernel_spmd(nc, _in_maps(inputs), list(range(8)), trace=True)
    return res.exec_time_ns


if __name__ == "__main__":
    rng = np.random.RandomState(0)
    ins = {
        "x": rng.randn(8, 1, H, W).astype(np.float32),
        "W1": rng.randn(CH, 1, 3, 3).astype(np.float32) * 0.1,
        "b1": np.zeros(CH, np.float32),
        "W2": rng.randn(CH, CH, 3, 3).astype(np.float32) * 0.05,
        "b2": np.zeros(CH, np.float32),
        "W3": rng.randn(C3, CH, 3, 3).astype(np.float32) * 0.05,
        "b3": np.zeros(C3, np.float32),
    }
    print(kernel(**ins).shape)


# revision 41
# speedup vs baseline: 1.1626x; 1.0037x over previous
"""NlmCNN (weight-predicting CNN + per-pixel 13x13 weighted sum) on 8 trn2 cores.

Sharding: data-parallel over batch (8 images -> 8 cores), weights replicated.

Per-core layout trick: output y is the conv stack's result center-cropped by
6 pixels, and the receptive field of the three 3x3 convs only reaches 3 px
out, so SAME-padding never materializes: every layer is computed VALID-style
on an unpadded 256-stride flat layout. Column-wrap junk from flat shifted
reads stays confined to the outer <=3 columns of each layer, which are
discarded by the crop.

All matmul operands are bf16 (fp32 PSUM accumulation; end-to-end absmax-rel
~4e-3 vs the 2e-2 gate). bf16 is chosen over float32r because fp32-class
LDWEIGHTS runs ~2.2ns/stationary-column with FWL disabled: M=128 weight
loads (285ns) exceed the N=512 stream time (213ns) and the PE becomes
weight-load-bound (measured 426ns/matmul cadence). bf16 enables FWL and
streams the same 1 column/cycle.

Pipeline per strip of S output rows (strips software-pipelined: conv1 of
strip i+1 is emitted during strip i's conv3 phase):
  conv1: per-2-chunk im2col [9, 1024] via one 3-dim DMA -> K=9 matmul; all
         of a strip's im2col DMAs are issued a strip ahead (the imc pool
         holds a full strip) so the PE never waits on DMA latency.
  conv2/conv3: 3x3 taps packed into K=128 pair-matmuls: taps (du,0)+(du,1)
         pair on hA=[h; h<<1] (upper 64 partitions hold h shifted +1);
         taps (0,2)+(1,2) pair on hB=[h; h<<W]; only tap (2,2) is a K=64
         single. 5 matmuls per 512-px chunk for conv2, 10 for conv3
         (out channels split [0:128] M=128 / [128:169] M=41). The shifted
         tiles are built by SBUF->SBUF dup DMAs per half-strip. Chunk
         parity alternates [single | pairs] / [pairs | single] so the PE
         sees one K-row-size transition per chunk.
  einsum: patch matrix xs[t, s] = x[pos + shift(t)] split [128 | 41] taps,
         gathered bf16 by one contiguous DMA per tap-row u (row u=9 is
         split 11/2 across the two tiles); DVE scalar_tensor_tensor
         computes t2 = (conv3_lo + b3_lo) * xs_lo and
         t_hi = (conv3_hi + b3_hi) * xs_hi straight from PSUM; the
         partition reduction is "staircase ones" matmuls (K=128 over t2
         next to the pairs, K=41 over t_hi next to the K=64 singles --
         both transition-free), accumulating 2-row chunk jj into row jj
         of a persistent PSUM tile; one copy + one DMA store the image.
"""

import numpy as np

import concourse.bacc as bacc
import concourse.bass as bass
import concourse.mybir as mybir
import concourse.tile as tile
from concourse.bass_utils import run_bass_kernel_spmd

F32 = mybir.dt.float32
BF16 = mybir.dt.bfloat16
AF = mybir.ActivationFunctionType
ALU = mybir.AluOpType

H = 256
W = 256
K = 13
HO = H - K + 1  # 244
CH = 64
C3 = K * K  # 169
CLO = 128   # conv3 out-channel group sizes
CHI = C3 - 128  # 41
S_STRIP = 16
NC_ = 512  # chunk positions (2 image rows)
import os
PIPE = os.environ.get("K_PIPE", "1") == "1"      # cross-strip sw pipelining
XS_GP = os.environ.get("K_XS_GP", "1") == "1"    # xs DMAs on gpsimd queue
DUP_GP = os.environ.get("K_DUP_GP", "1") == "1"  # dup DMAs on gpsimd queue


def _ap(t, off, dims):
    return bass.AP(t, off, [list(d) for d in dims])


def _mm(nc, out, lhsT, rhs, start, stop):
    nc.tensor.matmul(out, lhsT, rhs, start=start, stop=stop)


def build_nc():
    nc = bacc.Bacc("TRN2", target_bir_lowering=False, debug=False)

    x = nc.dram_tensor("x", [1, 1, H, W], F32, kind="ExternalInput")
    w1 = nc.dram_tensor("W1", [CH, 1, 3, 3], F32, kind="ExternalInput")
    b1 = nc.dram_tensor("b1", [CH], F32, kind="ExternalInput")
    w2 = nc.dram_tensor("W2", [CH, CH, 3, 3], F32, kind="ExternalInput")
    b2 = nc.dram_tensor("b2", [CH], F32, kind="ExternalInput")
    w3 = nc.dram_tensor("W3", [C3, CH, 3, 3], F32, kind="ExternalInput")
    b3 = nc.dram_tensor("b3", [C3], F32, kind="ExternalInput")
    y = nc.dram_tensor("y", [1, 1, HO, HO], F32, kind="ExternalOutput")
    xb = nc.dram_tensor("x_b", [H * W], BF16)

    with tile.TileContext(nc) as tc:
        Body(nc, tc, x, w1, b1, w2, b2, w3, b3, y, xb).build()

    nc.compile()
    return nc


class Body:
    def __init__(self, nc, tc, x, w1, b1, w2, b2, w3, b3, y, xb):
        self.nc, self.tc = nc, tc
        self.x, self.w1, self.b1, self.w2, self.b2 = x, w1, b1, w2, b2
        self.w3, self.b3, self.y, self.xb = w3, b3, y, xb

    def build(self):
        nc, tc = self.nc, self.tc
        with (
            tc.tile_pool(name="consts", bufs=1) as consts,
            tc.tile_pool(name="t2p", bufs=5) as p_t2,
            tc.tile_pool(name="thp", bufs=5) as p_th,
            tc.tile_pool(name="imc", bufs=6) as p_imc,
            tc.tile_pool(name="h1p", bufs=2) as p_h1,
            tc.tile_pool(name="h1bp", bufs=2) as p_h1b,
            tc.tile_pool(name="h2p", bufs=2) as p_h2,
            tc.tile_pool(name="h2bp", bufs=2) as p_h2b,
            tc.tile_pool(name="xsl", bufs=2) as p_xsl,
            tc.tile_pool(name="xsh", bufs=2) as p_xsh,
            tc.tile_pool(name="yout", bufs=1) as p_y,
            tc.tile_pool(name="ps12", bufs=3, space="PSUM") as ps12,
            tc.tile_pool(name="ps3", bufs=2, space="PSUM") as ps3,
            tc.tile_pool(name="psy", bufs=1, space="PSUM") as psy,
        ):
            self.consts = consts
            self.p_t2, self.p_th, self.p_imc = p_t2, p_th, p_imc
            self.p_h1, self.p_h1b = p_h1, p_h1b
            self.p_h2, self.p_h2b = p_h2, p_h2b
            self.p_xsl, self.p_xsh = p_xsl, p_xsh
            self.p_y, self.ps12, self.ps3, self.psy = p_y, ps12, ps3, psy
            self._build_consts()
            self._build_strips()

    def _build_consts(self):
        nc, tc, consts = self.nc, self.tc, self.consts
        stage = tc.alloc_tile_pool(name="stage", bufs=1)
        # weight-prep transposes borrow ps3's "ps3lo" slot (same max tile
        # size, consts-time only) so no dedicated PSUM bank is needed
        pwtr = self.ps3

        # Weights arrive [co, ci, du, dv]; matmuls need [ci, co] per tap.
        # A strided gather DMA would be 4-byte-descriptor-bound, so load
        # contiguously and transpose on the PE instead.
        from concourse.masks import make_identity

        ident = stage.tile([128, 128], F32)
        make_identity(nc, ident[:])

        # x -> bf16 copy in DRAM (conv1 im2col + xs gather source); staged
        # FIRST so strip 0's im2col/xs DMAs can start while weight prep runs
        xst = stage.tile([128, H * W // 128], F32)
        nc.sync.dma_start(
            out=xst[:], in_=_ap(self.x, 0, [(H * W // 128, 128), (1, H * W // 128)])
        )
        xsb = stage.tile([128, H * W // 128], BF16)
        nc.vector.tensor_copy(xsb[:], xst[:])
        nc.sync.dma_start(
            out=_ap(self.xb, 0, [(H * W // 128, 128), (1, H * W // 128)]), in_=xsb[:]
        )

        w1raw = stage.tile([CH, 9], F32)
        nc.scalar.dma_start(out=w1raw[:], in_=_ap(self.w1, 0, [(9, CH), (1, 9)]))
        w2raw = stage.tile([CH, 9 * CH], F32)
        nc.scalar.dma_start(out=w2raw[:], in_=_ap(self.w2, 0, [(9 * CH, CH), (1, 9 * CH)]))
        w3raw_a = stage.tile([128, 9 * CH], F32)
        nc.scalar.dma_start(
            out=w3raw_a[:], in_=_ap(self.w3, 0, [(9 * CH, 128), (1, 9 * CH)])
        )
        w3raw_b = stage.tile([CHI, 9 * CH], F32)
        nc.scalar.dma_start(
            out=w3raw_b[:],
            in_=_ap(self.w3, 128 * 9 * CH, [(9 * CH, CHI), (1, 9 * CH)]),
        )

        def tapv(raw, t, n):  # [n_co, ci] view of tap t
            return raw[0:n, :].rearrange("p (ci t) -> p t ci", t=9)[:, t, :]

        # All lhsT tiles are K=128-padded with ZERO rows so every matmul
        # shares the 128-row stationary config: the PE pays ~100ns whenever
        # consecutive matmuls change stationary geometry (row or column
        # count), and a K=128 matmul streams the same N columns as a K=9
        # one. Zero weight rows turn the junk in the corresponding rhs
        # partitions into exact zeros.
        # w1: lhsT rows 0-8 = taps, rows 9-127 = 0.
        pT = pwtr.tile([128, 128], F32, tag="ps3lo")
        nc.tensor.transpose(pT[0:9, 0:CH], w1raw[:], ident[0:CH, 0:CH])
        self.w1sb = consts.tile([128, 128], BF16)
        nc.vector.memset(self.w1sb[:], 0.0)
        nc.vector.tensor_copy(self.w1sb[0:9, 0:CH], pT[0:9, 0:CH])

        # Transpose each tap to PSUM base 0 (HW requires base 0); upper
        # (shifted-partner tap) halves staged then partition-shifted to
        # partitions 64-127 by one SBUF->SBUF DMA per weight tile.
        # A-pairs carry taps (du,0)+(du,1) du=0..2; B-pair (0,2)+(1,2);
        # single is tap (2,2); conv3 splits co into [0:128] / [128:169].
        # every lhsT is [128 x 128] (zero row/col padding): uniform
        # stationary geometry means the PE never pays a reconfiguration
        # drain, and NumWeights==128 keeps FWL on for every load
        self.w2p = consts.tile([128, 3 * 128], BF16)
        self.w2pB = consts.tile([128, 128], BF16)
        self.w2s = consts.tile([128, 128], BF16)
        self.w3pA_lo = consts.tile([128, 3 * CLO], BF16)
        self.w3pA_hi = consts.tile([128, 3 * 128], BF16)
        self.w3pB_lo = consts.tile([128, CLO], BF16)
        self.w3pB_hi = consts.tile([128, 128], BF16)
        self.w3s_lo = consts.tile([128, CLO], BF16)
        self.w3s_hi = consts.tile([128, 128], BF16)
        for wt in (self.w2p, self.w2pB, self.w2s, self.w3pA_hi,
                   self.w3pB_hi, self.w3s_lo, self.w3s_hi):
            nc.vector.memset(wt[:], 0.0)
        w2pu = stage.tile([CH, 3 * CH], BF16)
        w2puB = stage.tile([CH, CH], BF16)
        w3puA_lo = stage.tile([CH, 3 * CLO], BF16)
        w3puA_hi = stage.tile([CH, 3 * CHI], BF16)
        w3puB_lo = stage.tile([CH, CLO], BF16)
        w3puB_hi = stage.tile([CH, CHI], BF16)

        def tr(dst, raw, t, n):
            pT = pwtr.tile([CH, 128], F32, tag="ps3lo")
            nc.tensor.transpose(pT[:, 0:n], tapv(raw, t, n), ident[0:n, 0:n])
            nc.vector.tensor_copy(dst, pT[:, 0:n])

        for p in range(3):
            cw = slice(p * CH, (p + 1) * CH)
            cl = slice(p * CLO, (p + 1) * CLO)
            ch = slice(p * CHI, (p + 1) * CHI)
            cw2 = slice(p * 128, p * 128 + CH)
            ch2 = slice(p * 128, p * 128 + CHI)
            tr(self.w2p[0:CH, cw2], w2raw, p * 3, CH)
            tr(w2pu[:, cw], w2raw, p * 3 + 1, CH)
            tr(self.w3pA_lo[0:CH, cl], w3raw_a, p * 3, 128)
            tr(self.w3pA_hi[0:CH, ch2], w3raw_b, p * 3, CHI)
            tr(w3puA_lo[:, cl], w3raw_a, p * 3 + 1, 128)
            tr(w3puA_hi[:, ch], w3raw_b, p * 3 + 1, CHI)
        tr(self.w2pB[0:CH, 0:CH], w2raw, 2, CH)
        tr(w2puB[:], w2raw, 5, CH)
        tr(self.w2s[0:CH, 0:CH], w2raw, 8, CH)
        tr(self.w3pB_lo[0:CH, :], w3raw_a, 2, 128)
        tr(self.w3pB_hi[0:CH, 0:CHI], w3raw_b, 2, CHI)
        tr(w3puB_lo[:], w3raw_a, 5, 128)
        tr(w3puB_hi[:], w3raw_b, 5, CHI)
        tr(self.w3s_lo[0:CH, :], w3raw_a, 8, 128)
        tr(self.w3s_hi[0:CH, 0:CHI], w3raw_b, 8, CHI)
        for p in range(3):
            nc.scalar.dma_start(
                out=self.w2p[CH:, p * 128 : p * 128 + CH],
                in_=w2pu[:, p * CH : (p + 1) * CH])
            nc.scalar.dma_start(
                out=self.w3pA_hi[CH:, p * 128 : p * 128 + CHI],
                in_=w3puA_hi[:, p * CHI : (p + 1) * CHI])
        nc.scalar.dma_start(out=self.w2pB[CH:, 0:CH], in_=w2puB[:])
        nc.scalar.dma_start(out=self.w3pA_lo[CH:, :], in_=w3puA_lo[:])
        nc.scalar.dma_start(out=self.w3pB_lo[CH:, :], in_=w3puB_lo[:])
        nc.scalar.dma_start(out=self.w3pB_hi[CH:, 0:CHI], in_=w3puB_hi[:])

        # biases replicated into partitions 64-127 for the chunk-b relus
        # (engine lanes are partition-hardwired)
        self.b1sb = consts.tile([2 * CH, 1], F32)
        nc.scalar.dma_start(out=self.b1sb[0:CH], in_=_ap(self.b1, 0, [(1, CH), (0, 1)]))
        nc.scalar.dma_start(out=self.b1sb[CH:], in_=_ap(self.b1, 0, [(1, CH), (0, 1)]))
        self.b2sb = consts.tile([2 * CH, 1], F32)
        nc.scalar.dma_start(out=self.b2sb[0:CH], in_=_ap(self.b2, 0, [(1, CH), (0, 1)]))
        nc.scalar.dma_start(out=self.b2sb[CH:], in_=_ap(self.b2, 0, [(1, CH), (0, 1)]))
        self.b3lo = consts.tile([CLO, 1], F32)
        nc.scalar.dma_start(out=self.b3lo[:], in_=_ap(self.b3, 0, [(1, CLO), (0, 1)]))
        self.b3hi = consts.tile([CHI, 1], F32)
        nc.scalar.dma_start(out=self.b3hi[:], in_=_ap(self.b3, CLO, [(1, CHI), (0, 1)]))

        # staircase-ones: stair[:, 128] = 1, else 0; column j of the view
        # stair[:, 128-j : 192-j] is all-ones -> matmul writes the partition
        # sum into PSUM row j (zeros elsewhere, harmless under accumulation)
        stair_st = stage.tile([128, 256], F32)
        nc.vector.memset(stair_st[:], 0.0)
        nc.vector.memset(stair_st[:, 128:129], 1.0)
        self.stair = consts.tile([128, 256], BF16)
        nc.vector.tensor_copy(self.stair[:], stair_st[:])
        # hi-stair variant: ones only in rows 0..40 (t_hi's live taps) so a
        # K=128 matmul over the zero-padded t_hi reduces exactly 41 rows
        self.stair_h = consts.tile([128, 256], BF16)
        nc.vector.memset(self.stair_h[:], 0.0)
        nc.vector.tensor_copy(self.stair_h[0:CHI, :], stair_st[0:CHI, :])

        stage.release()

    # ---------------- per-strip stages ----------------

    def emit_conv1(self, i0, S, first_strip=False):
        # im2col DMAs for the whole strip are issued up-front (the 6-deep
        # imc pool holds a full strip) so the PE never waits on DMA latency
        # when the deferred matmul bursts run a strip later. imc tiles are
        # K=128-padded: rows 9..127 are zeroed once (first strip touches
        # every pool slot) and w1sb's zero rows make them inert.
        nc = self.nc
        c0 = i0 + 6
        L1 = (S + 6) * W
        L2 = (S + 3) * W
        LB = L2 + 320
        h1t = self.p_h1.tile([2 * CH, (S_STRIP + 6) * W + 772], BF16, tag="h1")
        h1b = self.p_h1b.tile([2 * CH, (S_STRIP + 4) * W + 320], BF16, tag="h1b")
        nc.gpsimd.memset(h1t[0:CH, L1 : L1 + 772], 0.0)
        nc.gpsimd.memset(h1t[CH:, L1 - 1 : L1 + 771], 0.0)
        Lh = (L1 // (2 * NC_)) * NC_
        groups = list(range(0, L1, 2 * NC_))
        imcs = {}

        def emit_dmas():
            for hs in groups:
                he = min(hs + 2 * NC_, L1)
                imc = self.p_imc.tile([128, 2 * NC_], BF16, tag="imc")
                if first_strip:
                    nc.gpsimd.memset(imc[:], 0.0)
                nc.sync.dma_start(
                    out=imc[0:9, 0 : he - hs],
                    in_=_ap(self.xb, (c0 - 5) * W - 1 + hs,
                            [(W, 3), (1, 3), (1, he - hs)]),
                )
                imcs[hs] = imc

        def emit_groups(grps):
            for hs in grps:
                he = min(hs + 2 * NC_, L1)
                imc = imcs[hs]
                for cs in range(hs, he, NC_):
                    ce = min(cs + NC_, L1)
                    pt = self.ps12.tile([128, NC_], F32, tag="ps12")
                    _mm(nc, pt[:, 0 : ce - cs], self.w1sb[:],
                        imc[:, cs - hs : ce - hs], True, True)
                    nc.scalar.activation(
                        h1t[0:CH, cs:ce], pt[0:CH, 0 : ce - cs], AF.Relu,
                        bias=self.b1sb[0:CH],
                    )
                    dup = nc.gpsimd if DUP_GP else nc.sync
                    if ce == Lh:
                        dup.dma_start(out=h1t[CH:, 0 : Lh - 1], in_=h1t[0:CH, 1:Lh])
                        dup.dma_start(out=h1b[0:CH, 0:Lh], in_=h1t[0:CH, 0:Lh])
                        dup.dma_start(out=h1b[CH:, 0 : Lh - W], in_=h1t[0:CH, W:Lh])
                    elif ce == L1:
                        dup.dma_start(
                            out=h1t[CH:, Lh - 1 : L1 - 1], in_=h1t[0:CH, Lh:L1]
                        )
                        dup.dma_start(out=h1b[0:CH, Lh:LB], in_=h1t[0:CH, Lh:LB])
                        dup.dma_start(
                            out=h1b[CH:, Lh - W : LB], in_=h1t[0:CH, Lh : LB + W]
                        )

        # split into three bursts so conv1's scalar-relu chain (684ns vs
        # ~290ns mm) doesn't back up the in-order PE queue in one long run
        return (h1t, h1b, emit_dmas, lambda: emit_groups(groups[:2]),
                lambda: emit_groups(groups[2:4]),
                lambda: emit_groups(groups[4:]))

    def emit_xs(self, i0, S):
        # xs[(u,v), i*W + j] = x[i0+u+i, j+v]: one contiguous read per
        # tap-row u (13 partitions x (S-1)*W+244 elements) into the spaced
        # layout; cols 244..256 of each row hold neighbor-row junk that the
        # stt views never touch. Tap-row u=9 straddles the 128-tap split:
        # taps 117..127 land in xs_lo[117:128], taps 128..129 in xs_hi[0:2].
        nc = self.nc
        LS = (S - 1) * W + HO
        xs_lo = self.p_xsl.tile([CLO, S_STRIP * W], BF16, tag="xsl")
        eng_lo = nc.gpsimd if XS_GP else nc.scalar
        eng_hi = nc.gpsimd if XS_GP else nc.sync
        for u in range(9):
            eng_lo.dma_start(
                out=xs_lo[u * K : (u + 1) * K, 0:LS],
                in_=_ap(self.xb, (i0 + u) * W, [(1, K), (1, LS)]),
            )
        eng_lo.dma_start(
            out=xs_lo[117:128, 0:LS],
            in_=_ap(self.xb, (i0 + 9) * W, [(1, 11), (1, LS)]),
        )
        xs_hi = self.p_xsh.tile([CHI, S_STRIP * W], BF16, tag="xsh")
        eng_hi.dma_start(
            out=xs_hi[0:2, 0:LS],
            in_=_ap(self.xb, (i0 + 9) * W + 11, [(1, 2), (1, LS)]),
        )
        for u in range(10, 13):
            eng_hi.dma_start(
                out=xs_hi[2 + (u - 10) * K : 2 + (u - 9) * K, 0:LS],
                in_=_ap(self.xb, (i0 + u) * W, [(1, K), (1, LS)]),
            )
        return xs_lo, xs_hi

    def emit_conv2(self, i0, S, h1t, h1b):
        # Chunk parity alternates [single K=64 | pairs K=128] and
        # [pairs | single] so same-row-size groups meet across chunk
        # boundaries: one PE row-size-transition drain per chunk.
        # Also builds the conv3 tiles h2t=[h2; h2<<1] / h2b=[h2; h2<<W]
        # via dup DMAs per half-strip.
        nc = self.nc
        L2 = (S + 3) * W
        LB = S * W + 320
        h2t = self.p_h2.tile([2 * CH, (S_STRIP + 3) * W + 772], BF16, tag="h2")
        h2b = self.p_h2b.tile([2 * CH, S_STRIP * W + 320], BF16, tag="h2b")
        nc.gpsimd.memset(h2t[0:CH, L2 : L2 + 772], 0.0)
        nc.gpsimd.memset(h2t[CH:, L2 - 1 : L2 + 771], 0.0)
        Lh = (L2 // (2 * NC_)) * NC_
        for ci, cs in enumerate(range(0, L2, NC_)):
            ce = min(cs + NC_, L2)
            pt = self.ps12.tile([128, NC_], F32, tag="ps12")

            for p in range(3):
                off = p * W + 255
                _mm(nc, pt[:, 0 : ce - cs],
                    self.w2p[:, p * 128 : (p + 1) * 128],
                    h1t[:, cs + off : ce + off], p == 0, False)
            offb = 2 + 255
            _mm(nc, pt[:, 0 : ce - cs], self.w2pB[:],
                h1b[:, cs + offb : ce + offb], False, False)
            offs = 2 * W + 2 + 255
            _mm(nc, pt[:, 0 : ce - cs], self.w2s[:],
                h1t[:, cs + offs : ce + offs], False, True)
            nc.scalar.activation(
                h2t[0:CH, cs:ce], pt[0:CH, 0 : ce - cs], AF.Relu,
                bias=self.b2sb[0:CH]
            )
            dup = nc.gpsimd if DUP_GP else nc.sync
            if ce == Lh:
                dup.dma_start(out=h2t[CH:, 0 : Lh - 1], in_=h2t[0:CH, 1:Lh])
                dup.dma_start(out=h2b[0:CH, 0:Lh], in_=h2t[0:CH, 0:Lh])
                dup.dma_start(out=h2b[CH:, 0 : Lh - W], in_=h2t[0:CH, W:Lh])
            elif ce == L2:
                dup.dma_start(out=h2t[CH:, Lh - 1 : L2 - 1], in_=h2t[0:CH, Lh:L2])
                dup.dma_start(out=h2b[0:CH, Lh:LB], in_=h2t[0:CH, Lh:LB])
                dup.dma_start(
                    out=h2b[CH:, Lh - W : LB], in_=h2t[0:CH, Lh : LB + W]
                )
        return h2t, h2b

    def emit_conv3_chunk(self, i0, cs, h2t, h2b, xs_lo, xs_hi, flush):
        """conv3 + stt for one 2-row chunk; staircase matmuls are deferred.

        Every matmul is K=128 (zero-padded weights for the lone single
        tap), so the only stationary-geometry changes are M: the M=128
        block (lo pairs+single, both stairs) and the M=41 block (hi
        pairs+single). Chunk parity mirrors the block order so chunk
        boundaries are transition-free: ONE geometry change per chunk."""
        nc = self.nc
        even = self.gchunk % 2 == 0
        self.gchunk += 1
        plo = self.ps3.tile([CLO, NC_], F32, tag="ps3lo")
        phi = self.ps3.tile([128, NC_], F32, tag="ps3hi")

        def block_lo():
            off = 2 * W + 2 + 255
            _mm(nc, plo[:], self.w3s_lo[:],
                h2t[:, cs + off : cs + NC_ + off], True, False)
            for p in range(3):
                off = p * W + 255
                _mm(nc, plo[:], self.w3pA_lo[:, p * CLO : (p + 1) * CLO],
                    h2t[:, cs + off : cs + NC_ + off], False, False)
            offb = 2 + 255
            _mm(nc, plo[:], self.w3pB_lo[:],
                h2b[:, cs + offb : cs + NC_ + offb], False, True)

        def block_hi():
            off = 2 * W + 2 + 255
            _mm(nc, phi[:], self.w3s_hi[:],
                h2t[:, cs + off : cs + NC_ + off], True, False)
            for p in range(3):
                off = p * W + 255
                _mm(nc, phi[:], self.w3pA_hi[:, p * 128 : (p + 1) * 128],
                    h2t[:, cs + off : cs + NC_ + off], False, False)
            offb = 2 + 255
            _mm(nc, phi[:], self.w3pB_hi[:],
                h2b[:, cs + offb : cs + NC_ + offb], False, True)

        flush_hi, flush_lo = flush
        if even:
            block_lo()
            flush_lo()
            flush_hi()
            block_hi()
        else:
            block_hi()
            flush_lo()
            flush_hi()
            block_lo()

        r2 = cs // W
        jj = (i0 + r2) // 2
        # t2 = (conv3_psum + b3) * xs, straight from PSUM on the DVE; all
        # three operands live in the 256-spaced [c, (r, col)] layout.
        # t_hi is K=128-padded: rows 41..127 are zeroed once per pool slot
        # (the first 5 chunks touch all 5 slots) and stair_h's zero rows
        # make them inert.
        t2 = self.p_t2.tile([CLO, NC_], BF16, tag="t2")
        t_hi = self.p_th.tile([128, NC_], BF16, tag="t_hi")
        if self.gchunk <= 5:
            nc.gpsimd.memset(t_hi[64:128, :], 0.0)
            nc.gpsimd.memset(t_hi[32:64, :], 0.0)
        wv_lo = plo[:].rearrange("p (r c) -> p r c", c=W)[:, :, 6 : 6 + HO]
        wv_hi = phi[0:CHI].rearrange("p (r c) -> p r c", c=W)[:, :, 6 : 6 + HO]
        xv_lo = xs_lo[:, cs : cs + NC_].rearrange("p (r c) -> p r c", c=W)[:, :, 0:HO]
        xv_hi = xs_hi[:, cs : cs + NC_].rearrange("p (r c) -> p r c", c=W)[:, :, 0:HO]
        tv_lo = t2[:].rearrange("p (r c) -> p r c", c=W)[:, :, 0:HO]
        tv_hi = t_hi[0:CHI].rearrange("p (r c) -> p r c", c=W)[:, :, 0:HO]
        nc.vector.scalar_tensor_tensor(
            out=tv_lo, in0=wv_lo, scalar=self.b3lo[:], in1=xv_lo,
            op0=ALU.add, op1=ALU.mult,
        )
        nc.vector.scalar_tensor_tensor(
            out=tv_hi, in0=wv_hi, scalar=self.b3hi[:], in1=xv_hi,
            op0=ALU.add, op1=ALU.mult,
        )
        self.pend_lo.append((t2, jj))
        self.pend_hi.append((t_hi, jj))

    def _stair_mm(self, t_t, stair, jj):
        # psum_y is one [128, 488] bank accumulating all 122 output chunks;
        # the staircase lhsT is K=128 x M=128 so both stairs share the lo
        # pairs' stationary geometry exactly (no PE reconfiguration drain).
        nc = self.nc
        rhs = t_t[:].rearrange("p (r c) -> p r c", c=W)[:, :, 0:HO]
        _mm(nc, self.psum_y[:], stair[:, 128 - jj : 256 - jj],
            rhs, self.cnt == 0, self.cnt == 2 * self.NYC - 1)
        self.cnt += 1

    def flush_stair_lo(self, keep=0):
        while len(self.pend_lo) > keep:
            t2, jj = self.pend_lo.pop(0)
            self._stair_mm(t2, self.stair, jj)

    def flush_stair_hi(self, keep=0):
        while len(self.pend_hi) > keep:
            t_hi, jj = self.pend_hi.pop(0)
            self._stair_mm(t_hi, self.stair_h, jj)

    def _build_strips(self):
        nc = self.nc
        self.NYC = (HO * HO) // 488  # 122
        self.psum_y = self.psy.tile([128, 488], F32, tag="y")
        self.pend_lo = []
        self.pend_hi = []
        self.cnt = 0
        self.gchunk = 0

        strips = []
        i0 = 0
        while i0 < HO:
            S = min(S_STRIP, HO - i0)
            # balance the tail: a tiny final strip concentrates dependency
            # stalls, so split the last 20 rows 10/10 instead of 16/4
            if HO - i0 == 20:
                S = 10
            strips.append((i0, S))
            i0 += S

        h1t, h1b, c1d, c1a, c1b, c1c = self.emit_conv1(*strips[0],
                                                       first_strip=True)
        c1d()
        c1a()
        c1b()
        c1c()
        xs = self.emit_xs(*strips[0])
        self.keep = 4 if PIPE else 0
        flush = (lambda: self.flush_stair_hi(keep=self.keep),
                 lambda: self.flush_stair_lo(keep=self.keep))
        for si, (i0, S) in enumerate(strips):
            if si == len(strips) - 1:
                self.keep = 0
            h2t, h2b = self.emit_conv2(i0, S, h1t, h1b)
            xs_lo, xs_hi = xs
            # prefetch next strip's xs while this strip's conv3 runs
            if si + 1 < len(strips):
                xs = self.emit_xs(*strips[si + 1])
                h1n, h1bn, c1d, c1a, c1b, c1c = self.emit_conv1(*strips[si + 1])
                c1d()
            else:
                c1a = c1b = c1c = None
            for ci, cs in enumerate(range(0, S * W, NC_)):
                self.emit_conv3_chunk(
                    i0, cs, h2t, h2b, xs_lo, xs_hi, flush=flush,
                )
                # overlap next strip's conv1 with this strip's conv3 tail,
                # split into three bursts
                if PIPE and ci == 1 and c1a is not None:
                    c1a()
                    c1a = None
                if PIPE and ci == 3 and c1b is not None:
                    c1b()
                    c1b = None
                if PIPE and ci == 5 and c1c is not None:
                    c1c()
                    c1c = None
            for fn in (c1a, c1b, c1c):
                if fn is not None:
                    fn()
            if si + 1 < len(strips):
                h1t, h1b = h1n, h1bn
            self.flush_stair_hi(keep=0)
            self.flush_stair_lo(keep=0)

        ysb = self.p_y.tile([self.NYC, 488], F32, tag="y")
        nc.vector.tensor_copy(ysb[:], self.psum_y[0 : self.NYC, :])
        nc.sync.dma_start(
            out=_ap(self.y, 0, [(488, self.NYC), (1, 488)]), in_=ysb[:]
        )


_NC_CACHE = {}


def _get_nc():
    if "nc" not in _NC_CACHE:
        _NC_CACHE["nc"] = build_nc()
    return _NC_CACHE["nc"]


def _in_maps(inputs):
    x = np.ascontiguousarray(np.asarray(inputs["x"], dtype=np.float32))
    names = ["W1", "b1", "W2", "b2", "W3", "b3"]
    ws = {n: np.ascontiguousarray(np.asarray(inputs[n], np.float32)) for n in names}
    maps = []
    for i in range(8):
        m = {"x": x[i : i + 1]}
        m.update(ws)
        maps.append(m)
    return maps


def kernel(**inputs):
    nc = _get_nc()
    res = run_bass_kernel_spmd(nc, _in_maps(inputs), list(range(8)))
    return np.concatenate([res.results[i]["y"] for i in range(8)], axis=0)


def profile(**inputs):
    nc = _get_nc()
    res = run_bass_kernel_spmd(nc, _in_maps(inputs), list(range(8)), trace=True)
    return res.exec_time_ns


if __name__ == "__main__":
    rng = np.random.RandomState(0)
    ins = {
        "x": rng.randn(8, 1, H, W).astype(np.float32),
        "W1": rng.randn(CH, 1, 3, 3).astype(np.float32) * 0.1,
        "b1": np.zeros(CH, np.float32),
        "W2": rng.randn(CH, CH, 3, 3).astype(np.float32) * 0.05,
        "b2": np.zeros(CH, np.float32),
        "W3": rng.randn(C3, CH, 3, 3).astype(np.float32) * 0.05,
        "b3": np.zeros(C3, np.float32),
    }
    print(kernel(**ins).shape)


# revision 45
# speedup vs baseline: 1.2018x; 1.0337x over previous
"""NlmCNN (weight-predicting CNN + per-pixel 13x13 weighted sum) on 8 trn2 cores.

Sharding: data-parallel over batch (8 images -> 8 cores), weights replicated.

Per-core layout trick: output y is the conv stack's result center-cropped by
6 pixels, and the receptive field of the three 3x3 convs only reaches 3 px
out, so SAME-padding never materializes: every layer is computed VALID-style
on an unpadded 256-stride flat layout. Column-wrap junk from flat shifted
reads stays confined to the outer <=3 columns of each layer, which are
discarded by the crop.

All matmul operands are bf16 (fp32 PSUM accumulation; end-to-end absmax-rel
~4e-3 vs the 2e-2 gate). bf16 is chosen over float32r because fp32-class
LDWEIGHTS runs ~2.2ns/stationary-column with FWL disabled: M=128 weight
loads (285ns) exceed the N=512 stream time (213ns) and the PE becomes
weight-load-bound (measured 426ns/matmul cadence). bf16 enables FWL and
streams the same 1 column/cycle.

Pipeline per strip of S output rows (strips software-pipelined: conv1 of
strip i+1 is emitted during strip i's conv3 phase):
  conv1: per-2-chunk im2col [9, 1024] via one 3-dim DMA -> K=9 matmul; all
         of a strip's im2col DMAs are issued a strip ahead (the imc pool
         holds a full strip) so the PE never waits on DMA latency.
  conv2/conv3: 3x3 taps packed into K=128 pair-matmuls: taps (du,0)+(du,1)
         pair on hA=[h; h<<1] (upper 64 partitions hold h shifted +1);
         taps (0,2)+(1,2) pair on hB=[h; h<<W]; only tap (2,2) is a K=64
         single. 5 matmuls per 512-px chunk for conv2, 10 for conv3
         (out channels split [0:128] M=128 / [128:169] M=41). The shifted
         tiles are built by SBUF->SBUF dup DMAs per half-strip. Chunk
         parity alternates [single | pairs] / [pairs | single] so the PE
         sees one K-row-size transition per chunk.
  einsum: patch matrix xs[t, s] = x[pos + shift(t)] split [128 | 41] taps,
         gathered bf16 by one contiguous DMA per tap-row u (row u=9 is
         split 11/2 across the two tiles); DVE scalar_tensor_tensor
         computes t2 = (conv3_lo + b3_lo) * xs_lo and
         t_hi = (conv3_hi + b3_hi) * xs_hi straight from PSUM; the
         partition reduction is "staircase ones" matmuls (K=128 over t2
         next to the pairs, K=41 over t_hi next to the K=64 singles --
         both transition-free), accumulating 2-row chunk jj into row jj
         of a persistent PSUM tile; one copy + one DMA store the image.
"""

import numpy as np

import concourse.bacc as bacc
import concourse.bass as bass
import concourse.mybir as mybir
import concourse.tile as tile
from concourse.bass_utils import run_bass_kernel_spmd

F32 = mybir.dt.float32
BF16 = mybir.dt.bfloat16
AF = mybir.ActivationFunctionType
ALU = mybir.AluOpType

H = 256
W = 256
K = 13
HO = H - K + 1  # 244
CH = 64
C3 = K * K  # 169
CLO = 128   # conv3 out-channel group sizes
CHI = C3 - 128  # 41
S_STRIP = 16
NC_ = 512  # chunk positions (2 image rows)
import os
PIPE = os.environ.get("K_PIPE", "1") == "1"      # cross-strip sw pipelining
XS_GP = os.environ.get("K_XS_GP", "1") == "1"    # xs DMAs on gpsimd queue
DUP_GP = os.environ.get("K_DUP_GP", "1") == "1"  # dup DMAs on gpsimd queue


def _ap(t, off, dims):
    return bass.AP(t, off, [list(d) for d in dims])


def _mm(nc, out, lhsT, rhs, start, stop):
    nc.tensor.matmul(out, lhsT, rhs, start=start, stop=stop)


def build_nc():
    nc = bacc.Bacc("TRN2", target_bir_lowering=False, debug=False)

    x = nc.dram_tensor("x", [1, 1, H, W], F32, kind="ExternalInput")
    w1 = nc.dram_tensor("W1", [CH, 1, 3, 3], F32, kind="ExternalInput")
    b1 = nc.dram_tensor("b1", [CH], F32, kind="ExternalInput")
    w2 = nc.dram_tensor("W2", [CH, CH, 3, 3], F32, kind="ExternalInput")
    b2 = nc.dram_tensor("b2", [CH], F32, kind="ExternalInput")
    w3 = nc.dram_tensor("W3", [C3, CH, 3, 3], F32, kind="ExternalInput")
    b3 = nc.dram_tensor("b3", [C3], F32, kind="ExternalInput")
    y = nc.dram_tensor("y", [1, 1, HO, HO], F32, kind="ExternalOutput")
    xb = nc.dram_tensor("x_b", [H * W], BF16)

    with tile.TileContext(nc) as tc:
        Body(nc, tc, x, w1, b1, w2, b2, w3, b3, y, xb).build()

    nc.compile()
    return nc


class Body:
    def __init__(self, nc, tc, x, w1, b1, w2, b2, w3, b3, y, xb):
        self.nc, self.tc = nc, tc
        self.x, self.w1, self.b1, self.w2, self.b2 = x, w1, b1, w2, b2
        self.w3, self.b3, self.y, self.xb = w3, b3, y, xb

    def build(self):
        nc, tc = self.nc, self.tc
        with (
            tc.tile_pool(name="consts", bufs=1) as consts,
            tc.tile_pool(name="t2p", bufs=5) as p_t2,
            tc.tile_pool(name="thp", bufs=5) as p_th,
            tc.tile_pool(name="imc", bufs=6) as p_imc,
            tc.tile_pool(name="h1p", bufs=2) as p_h1,
            tc.tile_pool(name="h1bp", bufs=2) as p_h1b,
            tc.tile_pool(name="h2p", bufs=2) as p_h2,
            tc.tile_pool(name="h2bp", bufs=2) as p_h2b,
            tc.tile_pool(name="xsl", bufs=2) as p_xsl,
            tc.tile_pool(name="xsh", bufs=2) as p_xsh,
            tc.tile_pool(name="yout", bufs=1) as p_y,
            tc.tile_pool(name="ps12", bufs=3, space="PSUM") as ps12,
            tc.tile_pool(name="ps3", bufs=2, space="PSUM") as ps3,
            tc.tile_pool(name="psy", bufs=1, space="PSUM") as psy,
        ):
            self.consts = consts
            self.p_t2, self.p_th, self.p_imc = p_t2, p_th, p_imc
            self.p_h1, self.p_h1b = p_h1, p_h1b
            self.p_h2, self.p_h2b = p_h2, p_h2b
            self.p_xsl, self.p_xsh = p_xsl, p_xsh
            self.p_y, self.ps12, self.ps3, self.psy = p_y, ps12, ps3, psy
            self._build_consts()
            self._build_strips()

    def _build_consts(self):
        nc, tc, consts = self.nc, self.tc, self.consts
        stage = tc.alloc_tile_pool(name="stage", bufs=1)
        # weight-prep transposes borrow ps3's "ps3lo" slot (same max tile
        # size, consts-time only) so no dedicated PSUM bank is needed
        pwtr = self.ps3

        # Weights arrive [co, ci, du, dv]; matmuls need [ci, co] per tap.
        # A strided gather DMA would be 4-byte-descriptor-bound, so load
        # contiguously and transpose on the PE instead.
        from concourse.masks import make_identity

        ident = stage.tile([128, 128], F32)
        make_identity(nc, ident[:])

        # x -> bf16 copy in DRAM (conv1 im2col + xs gather source); staged
        # FIRST so strip 0's im2col/xs DMAs can start while weight prep runs
        xst = stage.tile([128, H * W // 128], F32)
        nc.sync.dma_start(
            out=xst[:], in_=_ap(self.x, 0, [(H * W // 128, 128), (1, H * W // 128)])
        )
        xsb = stage.tile([128, H * W // 128], BF16)
        nc.vector.tensor_copy(xsb[:], xst[:])
        nc.sync.dma_start(
            out=_ap(self.xb, 0, [(H * W // 128, 128), (1, H * W // 128)]), in_=xsb[:]
        )

        w1raw = stage.tile([CH, 9], F32)
        nc.scalar.dma_start(out=w1raw[:], in_=_ap(self.w1, 0, [(9, CH), (1, 9)]))
        w2raw = stage.tile([CH, 9 * CH], F32)
        nc.scalar.dma_start(out=w2raw[:], in_=_ap(self.w2, 0, [(9 * CH, CH), (1, 9 * CH)]))
        w3raw_a = stage.tile([128, 9 * CH], F32)
        nc.scalar.dma_start(
            out=w3raw_a[:], in_=_ap(self.w3, 0, [(9 * CH, 128), (1, 9 * CH)])
        )
        w3raw_b = stage.tile([CHI, 9 * CH], F32)
        nc.scalar.dma_start(
            out=w3raw_b[:],
            in_=_ap(self.w3, 128 * 9 * CH, [(9 * CH, CHI), (1, 9 * CH)]),
        )

        def tapv(raw, t, n):  # [n_co, ci] view of tap t
            return raw[0:n, :].rearrange("p (ci t) -> p t ci", t=9)[:, t, :]

        # All lhsT tiles are K=128-padded with ZERO rows so every matmul
        # shares the 128-row stationary config: the PE pays ~100ns whenever
        # consecutive matmuls change stationary geometry (row or column
        # count), and a K=128 matmul streams the same N columns as a K=9
        # one. Zero weight rows turn the junk in the corresponding rhs
        # partitions into exact zeros.
        # w1: lhsT rows 0-8 = taps, rows 9-127 = 0.
        pT = pwtr.tile([128, 128], F32, tag="ps3lo")
        nc.tensor.transpose(pT[0:9, 0:CH], w1raw[:], ident[0:CH, 0:CH])
        self.w1sb = consts.tile([128, 128], BF16)
        nc.vector.memset(self.w1sb[:], 0.0)
        nc.vector.tensor_copy(self.w1sb[0:9, 0:CH], pT[0:9, 0:CH])

        # Transpose each tap to PSUM base 0 (HW requires base 0); upper
        # (shifted-partner tap) halves staged then partition-shifted to
        # partitions 64-127 by one SBUF->SBUF DMA per weight tile.
        # A-pairs carry taps (du,0)+(du,1) du=0..2; B-pair (0,2)+(1,2);
        # single is tap (2,2); conv3 splits co into [0:128] / [128:169].
        # every lhsT is [128 x 128] (zero row/col padding): uniform
        # stationary geometry means the PE never pays a reconfiguration
        # drain, and NumWeights==128 keeps FWL on for every load
        self.w2p = consts.tile([128, 3 * 128], BF16)
        self.w2pB = consts.tile([128, 128], BF16)
        self.w2s = consts.tile([128, 128], BF16)
        self.w3pA_lo = consts.tile([128, 3 * CLO], BF16)
        self.w3pA_hi = consts.tile([128, 3 * 128], BF16)
        self.w3pB_lo = consts.tile([128, CLO], BF16)
        self.w3pB_hi = consts.tile([128, 128], BF16)
        self.w3s_lo = consts.tile([128, CLO], BF16)
        self.w3s_hi = consts.tile([128, 128], BF16)
        for wt in (self.w2p, self.w2pB, self.w2s, self.w3pA_hi,
                   self.w3pB_hi, self.w3s_lo, self.w3s_hi):
            nc.vector.memset(wt[:], 0.0)
        w2pu = stage.tile([CH, 3 * CH], BF16)
        w2puB = stage.tile([CH, CH], BF16)
        w3puA_lo = stage.tile([CH, 3 * CLO], BF16)
        w3puA_hi = stage.tile([CH, 3 * CHI], BF16)
        w3puB_lo = stage.tile([CH, CLO], BF16)
        w3puB_hi = stage.tile([CH, CHI], BF16)

        def tr(dst, raw, t, n):
            pT = pwtr.tile([CH, 128], F32, tag="ps3lo")
            nc.tensor.transpose(pT[:, 0:n], tapv(raw, t, n), ident[0:n, 0:n])
            nc.vector.tensor_copy(dst, pT[:, 0:n])

        for p in range(3):
            cw = slice(p * CH, (p + 1) * CH)
            cl = slice(p * CLO, (p + 1) * CLO)
            ch = slice(p * CHI, (p + 1) * CHI)
            cw2 = slice(p * 128, p * 128 + CH)
            ch2 = slice(p * 128, p * 128 + CHI)
            tr(self.w2p[0:CH, cw2], w2raw, p * 3, CH)
            tr(w2pu[:, cw], w2raw, p * 3 + 1, CH)
            tr(self.w3pA_lo[0:CH, cl], w3raw_a, p * 3, 128)
            tr(self.w3pA_hi[0:CH, ch2], w3raw_b, p * 3, CHI)
            tr(w3puA_lo[:, cl], w3raw_a, p * 3 + 1, 128)
            tr(w3puA_hi[:, ch], w3raw_b, p * 3 + 1, CHI)
        tr(self.w2pB[0:CH, 0:CH], w2raw, 2, CH)
        tr(w2puB[:], w2raw, 5, CH)
        tr(self.w2s[0:CH, 0:CH], w2raw, 8, CH)
        tr(self.w3pB_lo[0:CH, :], w3raw_a, 2, 128)
        tr(self.w3pB_hi[0:CH, 0:CHI], w3raw_b, 2, CHI)
        tr(w3puB_lo[:], w3raw_a, 5, 128)
        tr(w3puB_hi[:], w3raw_b, 5, CHI)
        tr(self.w3s_lo[0:CH, :], w3raw_a, 8, 128)
        tr(self.w3s_hi[0:CH, 0:CHI], w3raw_b, 8, CHI)
        for p in range(3):
            nc.scalar.dma_start(
                out=self.w2p[CH:, p * 128 : p * 128 + CH],
                in_=w2pu[:, p * CH : (p + 1) * CH])
            nc.scalar.dma_start(
                out=self.w3pA_hi[CH:, p * 128 : p * 128 + CHI],
                in_=w3puA_hi[:, p * CHI : (p + 1) * CHI])
        nc.scalar.dma_start(out=self.w2pB[CH:, 0:CH], in_=w2puB[:])
        nc.scalar.dma_start(out=self.w3pA_lo[CH:, :], in_=w3puA_lo[:])
        nc.scalar.dma_start(out=self.w3pB_lo[CH:, :], in_=w3puB_lo[:])
        nc.scalar.dma_start(out=self.w3pB_hi[CH:, 0:CHI], in_=w3puB_hi[:])

        # biases replicated into partitions 64-127 for the chunk-b relus
        # (engine lanes are partition-hardwired)
        self.b1sb = consts.tile([2 * CH, 1], F32)
        nc.scalar.dma_start(out=self.b1sb[0:CH], in_=_ap(self.b1, 0, [(1, CH), (0, 1)]))
        nc.scalar.dma_start(out=self.b1sb[CH:], in_=_ap(self.b1, 0, [(1, CH), (0, 1)]))
        self.b2sb = consts.tile([2 * CH, 1], F32)
        nc.scalar.dma_start(out=self.b2sb[0:CH], in_=_ap(self.b2, 0, [(1, CH), (0, 1)]))
        nc.scalar.dma_start(out=self.b2sb[CH:], in_=_ap(self.b2, 0, [(1, CH), (0, 1)]))
        self.b3lo = consts.tile([CLO, 1], F32)
        nc.scalar.dma_start(out=self.b3lo[:], in_=_ap(self.b3, 0, [(1, CLO), (0, 1)]))
        self.b3hi = consts.tile([CHI, 1], F32)
        nc.scalar.dma_start(out=self.b3hi[:], in_=_ap(self.b3, CLO, [(1, CHI), (0, 1)]))

        # staircase-ones: stair[:, 128] = 1, else 0; column j of the view
        # stair[:, 128-j : 192-j] is all-ones -> matmul writes the partition
        # sum into PSUM row j (zeros elsewhere, harmless under accumulation)
        stair_st = stage.tile([128, 256], F32)
        nc.vector.memset(stair_st[:], 0.0)
        nc.vector.memset(stair_st[:, 128:129], 1.0)
        self.stair = consts.tile([128, 256], BF16)
        nc.vector.tensor_copy(self.stair[:], stair_st[:])
        # hi-stair variant: ones only in rows 0..40 (t_hi's live taps) so a
        # K=128 matmul over the zero-padded t_hi reduces exactly 41 rows
        self.stair_h = consts.tile([128, 256], BF16)
        nc.vector.memset(self.stair_h[:], 0.0)
        nc.vector.tensor_copy(self.stair_h[0:CHI, :], stair_st[0:CHI, :])

        stage.release()

    # ---------------- per-strip stages ----------------

    def emit_conv1(self, i0, S, first_strip=False, prev=None):
        # im2col DMAs for the whole strip are issued up-front (the 6-deep
        # imc pool holds a full strip) so the PE never waits on DMA latency
        # when the deferred matmul bursts run a strip later. imc tiles are
        # K=128-padded: rows 9..127 are zeroed once (first strip touches
        # every pool slot) and w1sb's zero rows make them inert.
        #
        # Rolling: each strip's h1 overlaps the previous strip's last 6
        # rows; they are head-COPIED from the previous tile (one small DMA)
        # instead of recomputed, and only S fresh rows hit the PE. The
        # upper-half / B-tile dups regenerate everything from the lower
        # half, so only the lower head needs copying.
        nc = self.nc
        c0 = i0 + 6
        L1 = (S + 6) * W
        L2 = (S + 3) * W
        LB = L2 + 320
        h1t = self.p_h1.tile([2 * CH, (S_STRIP + 6) * W + 772], BF16, tag="h1")
        h1b = self.p_h1b.tile([2 * CH, (S_STRIP + 4) * W + 320], BF16, tag="h1b")
        nc.gpsimd.memset(h1t[0:CH, L1 : L1 + 772], 0.0)
        nc.gpsimd.memset(h1t[CH:, L1 - 1 : L1 + 771], 0.0)
        start = 0
        if prev is not None:
            h1prev, sprev = prev
            nc.sync.dma_start(
                out=h1t[0:CH, 0 : 6 * W],
                in_=h1prev[0:CH, sprev * W : (sprev + 6) * W],
            )
            start = 6 * W
        ends = [min(cs + NC_, L1) for cs in range(start, L1, NC_)]
        Lh = ends[len(ends) // 2 - 1]
        groups = list(range(start, L1, 2 * NC_))
        imcs = {}

        def emit_dmas():
            for hs in groups:
                he = min(hs + 2 * NC_, L1)
                imc = self.p_imc.tile([128, 2 * NC_], BF16, tag="imc")
                if first_strip:
                    nc.gpsimd.memset(imc[:], 0.0)
                nc.sync.dma_start(
                    out=imc[0:9, 0 : he - hs],
                    in_=_ap(self.xb, (c0 - 5) * W - 1 + hs,
                            [(W, 3), (1, 3), (1, he - hs)]),
                )
                imcs[hs] = imc

        def emit_groups(grps):
            for hs in grps:
                he = min(hs + 2 * NC_, L1)
                imc = imcs[hs]
                for cs in range(hs, he, NC_):
                    ce = min(cs + NC_, L1)
                    pt = self.ps12.tile([128, NC_], F32, tag="ps12")
                    _mm(nc, pt[:, 0 : ce - cs], self.w1sb[:],
                        imc[:, cs - hs : ce - hs], True, True)
                    nc.scalar.activation(
                        h1t[0:CH, cs:ce], pt[0:CH, 0 : ce - cs], AF.Relu,
                        bias=self.b1sb[0:CH],
                    )
                    dup = nc.gpsimd if DUP_GP else nc.sync
                    if ce == Lh:
                        dup.dma_start(out=h1t[CH:, 0 : Lh - 1], in_=h1t[0:CH, 1:Lh])
                        dup.dma_start(out=h1b[0:CH, 0:Lh], in_=h1t[0:CH, 0:Lh])
                        dup.dma_start(out=h1b[CH:, 0 : Lh - W], in_=h1t[0:CH, W:Lh])
                    elif ce == L1:
                        dup.dma_start(
                            out=h1t[CH:, Lh - 1 : L1 - 1], in_=h1t[0:CH, Lh:L1]
                        )
                        dup.dma_start(out=h1b[0:CH, Lh:LB], in_=h1t[0:CH, Lh:LB])
                        dup.dma_start(
                            out=h1b[CH:, Lh - W : LB], in_=h1t[0:CH, Lh : LB + W]
                        )

        # split into three bursts so conv1's scalar-relu chain (684ns vs
        # ~290ns mm) doesn't back up the in-order PE queue in one long run
        return (h1t, h1b, emit_dmas, lambda: emit_groups(groups[:2]),
                lambda: emit_groups(groups[2:3]),
                lambda: emit_groups(groups[3:]))

    def emit_xs(self, i0, S):
        # xs[(u,v), i*W + j] = x[i0+u+i, j+v]: one contiguous read per
        # tap-row u (13 partitions x (S-1)*W+244 elements) into the spaced
        # layout; cols 244..256 of each row hold neighbor-row junk that the
        # stt views never touch. Tap-row u=9 straddles the 128-tap split:
        # taps 117..127 land in xs_lo[117:128], taps 128..129 in xs_hi[0:2].
        nc = self.nc
        LS = (S - 1) * W + HO
        xs_lo = self.p_xsl.tile([CLO, S_STRIP * W], BF16, tag="xsl")
        eng_lo = nc.gpsimd if XS_GP else nc.scalar
        eng_hi = nc.gpsimd if XS_GP else nc.sync
        for u in range(9):
            eng_lo.dma_start(
                out=xs_lo[u * K : (u + 1) * K, 0:LS],
                in_=_ap(self.xb, (i0 + u) * W, [(1, K), (1, LS)]),
            )
        eng_lo.dma_start(
            out=xs_lo[117:128, 0:LS],
            in_=_ap(self.xb, (i0 + 9) * W, [(1, 11), (1, LS)]),
        )
        xs_hi = self.p_xsh.tile([CHI, S_STRIP * W], BF16, tag="xsh")
        eng_hi.dma_start(
            out=xs_hi[0:2, 0:LS],
            in_=_ap(self.xb, (i0 + 9) * W + 11, [(1, 2), (1, LS)]),
        )
        for u in range(10, 13):
            eng_hi.dma_start(
                out=xs_hi[2 + (u - 10) * K : 2 + (u - 9) * K, 0:LS],
                in_=_ap(self.xb, (i0 + u) * W, [(1, K), (1, LS)]),
            )
        return xs_lo, xs_hi

    def emit_conv2(self, i0, S, h1t, h1b, prev=None):
        # Chunk parity alternates [single K=64 | pairs K=128] and
        # [pairs | single] so same-row-size groups meet across chunk
        # boundaries: one PE row-size-transition drain per chunk.
        # Also builds the conv3 tiles h2t=[h2; h2<<1] / h2b=[h2; h2<<W]
        # via dup DMAs per half-strip.
        nc = self.nc
        L2 = (S + 3) * W
        LB = S * W + 320
        h2t = self.p_h2.tile([2 * CH, (S_STRIP + 3) * W + 772], BF16, tag="h2")
        h2b = self.p_h2b.tile([2 * CH, S_STRIP * W + 320], BF16, tag="h2b")
        nc.gpsimd.memset(h2t[0:CH, L2 : L2 + 772], 0.0)
        nc.gpsimd.memset(h2t[CH:, L2 - 1 : L2 + 771], 0.0)
        start = 0
        if prev is not None:
            h2prev, sprev = prev
            nc.sync.dma_start(
                out=h2t[0:CH, 0 : 3 * W],
                in_=h2prev[0:CH, sprev * W : (sprev + 3) * W],
            )
            start = 3 * W
        ends = [min(cs + NC_, L2) for cs in range(start, L2, NC_)]
        Lh = ends[len(ends) // 2 - 1]
        for ci, cs in enumerate(range(start, L2, NC_)):
            ce = min(cs + NC_, L2)
            pt = self.ps12.tile([128, NC_], F32, tag="ps12")

            for p in range(3):
                off = p * W + 255
                _mm(nc, pt[:, 0 : ce - cs],
                    self.w2p[:, p * 128 : (p + 1) * 128],
                    h1t[:, cs + off : ce + off], p == 0, False)
            offb = 2 + 255
            _mm(nc, pt[:, 0 : ce - cs], self.w2pB[:],
                h1b[:, cs + offb : ce + offb], False, False)
            offs = 2 * W + 2 + 255
            _mm(nc, pt[:, 0 : ce - cs], self.w2s[:],
                h1t[:, cs + offs : ce + offs], False, True)
            nc.scalar.activation(
                h2t[0:CH, cs:ce], pt[0:CH, 0 : ce - cs], AF.Relu,
                bias=self.b2sb[0:CH]
            )
            dup = nc.gpsimd if DUP_GP else nc.sync
            if ce == Lh:
                dup.dma_start(out=h2t[CH:, 0 : Lh - 1], in_=h2t[0:CH, 1:Lh])
                dup.dma_start(out=h2b[0:CH, 0:Lh], in_=h2t[0:CH, 0:Lh])
                dup.dma_start(out=h2b[CH:, 0 : Lh - W], in_=h2t[0:CH, W:Lh])
            elif ce == L2:
                dup.dma_start(out=h2t[CH:, Lh - 1 : L2 - 1], in_=h2t[0:CH, Lh:L2])
                dup.dma_start(out=h2b[0:CH, Lh:LB], in_=h2t[0:CH, Lh:LB])
                dup.dma_start(
                    out=h2b[CH:, Lh - W : LB], in_=h2t[0:CH, Lh : LB + W]
                )
        return h2t, h2b

    def emit_conv3_chunk(self, i0, cs, h2t, h2b, xs_lo, xs_hi, flush):
        """conv3 + stt for one 2-row chunk; staircase matmuls are deferred.

        Every matmul is K=128 (zero-padded weights for the lone single
        tap), so the only stationary-geometry changes are M: the M=128
        block (lo pairs+single, both stairs) and the M=41 block (hi
        pairs+single). Chunk parity mirrors the block order so chunk
        boundaries are transition-free: ONE geometry change per chunk."""
        nc = self.nc
        even = self.gchunk % 2 == 0
        self.gchunk += 1
        plo = self.ps3.tile([CLO, NC_], F32, tag="ps3lo")
        phi = self.ps3.tile([128, NC_], F32, tag="ps3hi")

        def block_lo():
            off = 2 * W + 2 + 255
            _mm(nc, plo[:], self.w3s_lo[:],
                h2t[:, cs + off : cs + NC_ + off], True, False)
            for p in range(3):
                off = p * W + 255
                _mm(nc, plo[:], self.w3pA_lo[:, p * CLO : (p + 1) * CLO],
                    h2t[:, cs + off : cs + NC_ + off], False, False)
            offb = 2 + 255
            _mm(nc, plo[:], self.w3pB_lo[:],
                h2b[:, cs + offb : cs + NC_ + offb], False, True)

        def block_hi():
            off = 2 * W + 2 + 255
            _mm(nc, phi[:], self.w3s_hi[:],
                h2t[:, cs + off : cs + NC_ + off], True, False)
            for p in range(3):
                off = p * W + 255
                _mm(nc, phi[:], self.w3pA_hi[:, p * 128 : (p + 1) * 128],
                    h2t[:, cs + off : cs + NC_ + off], False, False)
            offb = 2 + 255
            _mm(nc, phi[:], self.w3pB_hi[:],
                h2b[:, cs + offb : cs + NC_ + offb], False, True)

        flush_hi, flush_lo = flush
        if even:
            block_lo()
            flush_lo()
            flush_hi()
            block_hi()
        else:
            block_hi()
            flush_lo()
            flush_hi()
            block_lo()

        r2 = cs // W
        jj = (i0 + r2) // 2
        # t2 = (conv3_psum + b3) * xs, straight from PSUM on the DVE; all
        # three operands live in the 256-spaced [c, (r, col)] layout.
        # t_hi is K=128-padded: rows 41..127 are zeroed once per pool slot
        # (the first 5 chunks touch all 5 slots) and stair_h's zero rows
        # make them inert.
        t2 = self.p_t2.tile([CLO, NC_], BF16, tag="t2")
        t_hi = self.p_th.tile([128, NC_], BF16, tag="t_hi")
        if self.gchunk <= 5:
            nc.gpsimd.memset(t_hi[64:128, :], 0.0)
            nc.gpsimd.memset(t_hi[32:64, :], 0.0)
        wv_lo = plo[:].rearrange("p (r c) -> p r c", c=W)[:, :, 6 : 6 + HO]
        wv_hi = phi[0:CHI].rearrange("p (r c) -> p r c", c=W)[:, :, 6 : 6 + HO]
        xv_lo = xs_lo[:, cs : cs + NC_].rearrange("p (r c) -> p r c", c=W)[:, :, 0:HO]
        xv_hi = xs_hi[:, cs : cs + NC_].rearrange("p (r c) -> p r c", c=W)[:, :, 0:HO]
        tv_lo = t2[:].rearrange("p (r c) -> p r c", c=W)[:, :, 0:HO]
        tv_hi = t_hi[0:CHI].rearrange("p (r c) -> p r c", c=W)[:, :, 0:HO]
        nc.vector.scalar_tensor_tensor(
            out=tv_lo, in0=wv_lo, scalar=self.b3lo[:], in1=xv_lo,
            op0=ALU.add, op1=ALU.mult,
        )
        nc.vector.scalar_tensor_tensor(
            out=tv_hi, in0=wv_hi, scalar=self.b3hi[:], in1=xv_hi,
            op0=ALU.add, op1=ALU.mult,
        )
        self.pend_lo.append((t2, jj))
        self.pend_hi.append((t_hi, jj))

    def _stair_mm(self, t_t, stair, jj):
        # psum_y is one [128, 488] bank accumulating all 122 output chunks;
        # the staircase lhsT is K=128 x M=128 so both stairs share the lo
        # pairs' stationary geometry exactly (no PE reconfiguration drain).
        nc = self.nc
        rhs = t_t[:].rearrange("p (r c) -> p r c", c=W)[:, :, 0:HO]
        _mm(nc, self.psum_y[:], stair[:, 128 - jj : 256 - jj],
            rhs, self.cnt == 0, self.cnt == 2 * self.NYC - 1)
        self.cnt += 1

    def flush_stair_lo(self, keep=0):
        while len(self.pend_lo) > keep:
            t2, jj = self.pend_lo.pop(0)
            self._stair_mm(t2, self.stair, jj)

    def flush_stair_hi(self, keep=0):
        while len(self.pend_hi) > keep:
            t_hi, jj = self.pend_hi.pop(0)
            self._stair_mm(t_hi, self.stair_h, jj)

    def _build_strips(self):
        nc = self.nc
        self.NYC = (HO * HO) // 488  # 122
        self.psum_y = self.psy.tile([128, 488], F32, tag="y")
        self.pend_lo = []
        self.pend_hi = []
        self.cnt = 0
        self.gchunk = 0

        strips = []
        i0 = 0
        while i0 < HO:
            S = min(S_STRIP, HO - i0)
            # balance the tail: a tiny final strip concentrates dependency
            # stalls, so split the last 20 rows 10/10 instead of 16/4
            if HO - i0 == 20:
                S = 10
            strips.append((i0, S))
            i0 += S

        h1t, h1b, c1d, c1a, c1b, c1c = self.emit_conv1(*strips[0],
                                                       first_strip=True)
        c1d()
        c1a()
        c1b()
        c1c()
        xs = self.emit_xs(*strips[0])
        self.keep = 4 if PIPE else 0
        flush = (lambda: self.flush_stair_hi(keep=self.keep),
                 lambda: self.flush_stair_lo(keep=self.keep))
        h2prev = None
        for si, (i0, S) in enumerate(strips):
            if si == len(strips) - 1:
                self.keep = 0
            h2t, h2b = self.emit_conv2(i0, S, h1t, h1b, prev=h2prev)
            h2prev = (h2t, S)
            xs_lo, xs_hi = xs
            # prefetch next strip's xs while this strip's conv3 runs
            if si + 1 < len(strips):
                xs = self.emit_xs(*strips[si + 1])
                h1n, h1bn, c1d, c1a, c1b, c1c = self.emit_conv1(
                    *strips[si + 1], prev=(h1t, S))
                c1d()
            else:
                c1a = c1b = c1c = None
            for ci, cs in enumerate(range(0, S * W, NC_)):
                self.emit_conv3_chunk(
                    i0, cs, h2t, h2b, xs_lo, xs_hi, flush=flush,
                )
                # overlap next strip's conv1 with this strip's conv3 tail,
                # split into three bursts
                if PIPE and ci == 1 and c1a is not None:
                    c1a()
                    c1a = None
                if PIPE and ci == 3 and c1b is not None:
                    c1b()
                    c1b = None
                if PIPE and ci == 5 and c1c is not None:
                    c1c()
                    c1c = None
            for fn in (c1a, c1b, c1c):
                if fn is not None:
                    fn()
            if si + 1 < len(strips):
                h1t, h1b = h1n, h1bn
            self.flush_stair_hi(keep=0)
            self.flush_stair_lo(keep=0)

        ysb = self.p_y.tile([self.NYC, 488], F32, tag="y")
        nc.vector.tensor_copy(ysb[:], self.psum_y[0 : self.NYC, :])
        nc.sync.dma_start(
            out=_ap(self.y, 0, [(488, self.NYC), (1, 488)]), in_=ysb[:]
        )


_NC_CACHE = {}


def _get_nc():
    if "nc" not in _NC_CACHE:
        _NC_CACHE["nc"] = build_nc()
    return _NC_CACHE["nc"]


def _in_maps(inputs):
    x = np.ascontiguousarray(np.asarray(inputs["x"], dtype=np.float32))
    names = ["W1", "b1", "W2", "b2", "W3", "b3"]
    ws = {n: np.ascontiguousarray(np.asarray(inputs[n], np.float32)) for n in names}
    maps = []
    for i in range(8):
        m = {"x": x[i : i + 1]}
        m.update(ws)
        maps.append(m)
    return maps


def kernel(**inputs):
    nc = _get_nc()
    res = run_bass_kernel_spmd(nc, _in_maps(inputs), list(range(8)))
    return np.concatenate([res.results[i]["y"] for i in range(8)], axis=0)


def profile(**inputs):
    nc = _get_nc()
    res = run_bass_kernel_spmd(nc, _in_maps(inputs), list(range(8)), trace=True)
    return res.exec_time_ns


if __name__ == "__main__":
    rng = np.random.RandomState(0)
    ins = {
        "x": rng.randn(8, 1, H, W).astype(np.float32),
        "W1": rng.randn(CH, 1, 3, 3).astype(np.float32) * 0.1,
        "b1": np.zeros(CH, np.float32),
        "W2": rng.randn(CH, CH, 3, 3).astype(np.float32) * 0.05,
        "b2": np.zeros(CH, np.float32),
        "W3": rng.randn(C3, CH, 3, 3).astype(np.float32) * 0.05,
        "b3": np.zeros(C3, np.float32),
    }
    print(kernel(**ins).shape)
